# Initial kernel scaffold
#
"""Trainium2 Bass kernel for nn_Backbone_47390669144486 (SAM-style 4D-correlation attention).

Strategy: data-parallel over the 75 queries across 8 NeuronCores (pad to 80,
10 queries/core).  The entire per-query pipeline (1x1 conv + BN + ReLU,
L2-normalize, 400x400 correlation per way, dual gauss-norm softmax branches,
attention pooling, cosine similarity) is fused on-chip; nothing but the inputs
and the final [way, nq] similarities touch HBM.

Key algebraic moves (all exact up to fp rounding):
  * softmax is shift-invariant -> the gauss-norm mean never needs to be
    subtracted inside exp; only the scale r = 1/(TEMP*sqrt(var+eps)) matters.
  * the group variance factors through a CENTERED Gram matrix:
    corr[ij,kl] - mean_ij = s_ij . (q_kl - qbar), so
    sum_kl (corr - mean)^2 = s_ij^T Gc_q s_ij with
    Gc_q = sum_kl (q_kl - qbar)(q_kl - qbar)^T.  No S1/S2 cancellation, so
    bf16 inputs are numerically safe (the stats normalize the same quantized
    correlation matrix the exp sees).
  * the correlation matmul's moving operand is extended with the opposing
    feature's centered Gram: rhs = [h | Gc] (464 columns, one PSUM bank).
    One matmul per 100-position chunk yields the correlation block AND
    U = chunk^T Gc, whose row-dot with the transposed features (DVE
    accumulate) is the group variance.
  * exp + softmax-denominator fuse into one ScalarE activation (accum_out)
    for branch A; branch B's denominators run on the otherwise-idle GpSimd
    engine to unload ScalarE (the bottleneck).
  * attention sums use a stride-0 (free-dim broadcast) stationary operand
    built directly from the 1/S weight vector.
  * the /400 means and the 1e-6 norm clamps rescale out of the cosine
    similarity (clamp becomes 4e-4 on the unscaled norms).

Performance notes:
  * every hot-loop matmul runs bf16 (1 col/cycle AND keeps the PE HAM
    activity monitor warm at 2.4 GHz -- fp32r runs throttled at 1.2 GHz).
  * softmax numerators are written as bf16 by the exp activation.
  * emission is software-pipelined across the 50 (query, way) pairs so the
    scalar engine never waits at a pair boundary.
"""

import os
import sys

sys.path.insert(0, "/opt/trn_rl_repo")

import numpy as np

import concourse.bass as bass
import concourse.tile as tile
from concourse import bacc, masks, mybir
from concourse.bass_utils import run_bass_kernel_spmd

F32 = mybir.dt.float32
BF16 = mybir.dt.bfloat16
AF = mybir.ActivationFunctionType
OP = mybir.AluOpType
AX = mybir.AxisListType

WAY = 5
C = 64
S = 400          # 20*20 spatial positions
CH = 100         # chunk of the spatial dim that fits PSUM partitions
NCH = S // CH    # 4
EXT = S + C      # 464: [h | centered Gram] extended moving operand
NCORES = 8
QPC = 10         # queries per core (75 padded to 80)
TEMP = 5.0


def _build_program():
    nc = bacc.Bacc("TRN2", target_bir_lowering=False, debug=False)

    spt_t = nc.dram_tensor("spt", [WAY, C, S], F32, kind="ExternalInput")
    qry_t = nc.dram_tensor("qry", [QPC, C, S], F32, kind="ExternalInput")
    w_t = nc.dram_tensor("conv_w", [C, C], F32, kind="ExternalInput")
    gam_t = nc.dram_tensor("bn_gamma", [C], F32, kind="ExternalInput")
    bet_t = nc.dram_tensor("bn_beta", [C], F32, kind="ExternalInput")
    mu_t = nc.dram_tensor("bn_mean", [C], F32, kind="ExternalInput")
    var_t = nc.dram_tensor("bn_var", [C], F32, kind="ExternalInput")
    scl_t = nc.dram_tensor("scale", [1], F32, kind="ExternalInput")
    out_t = nc.dram_tensor("out_sim", [WAY, QPC], F32, kind="ExternalOutput")

    from contextlib import ExitStack

    with tile.TileContext(nc) as tc, ExitStack() as ctx:
        _emit(ctx, tc, nc, spt_t.ap(), qry_t.ap(), w_t.ap(), gam_t.ap(),
              bet_t.ap(), mu_t.ap(), var_t.ap(), scl_t.ap(), out_t.ap())
    nc.compile()
    _dedup_act_table_loads(nc)
    return nc


def _dedup_act_table_loads(nc):
    """The act-table pass alternates natural_log / exp_and_others per function.
    natural_log_exp_and_others serves every activation this kernel uses
    (Exp, Ln, Relu), so keep one load targeting it and drop the rest."""
    from concourse.hw_specs import get_activation_tables

    names = list(get_activation_tables(nc.m.arch).keys())
    combined = names.index("natural_log_exp_and_others")
    kept = False
    for b in nc.m.functions[0].blocks:
        keep = []
        for i in b.instructions:
            if type(i).__name__ == "InstLoadActFuncSet":
                si = i.sync_info
                assert si is None or (not si.on_wait and not si.on_update)
                if kept:
                    continue
                i.act_func_set_id = combined
                kept = True
            keep.append(i)
        if len(keep) != len(b.instructions):
            b.instructions[:] = keep


def _bcast_col(t_ap):
    """[P,1] AP -> [P,C] AP with free-dim stride 0 (partition-broadcast
    stationary operand for the attention-sum matmuls)."""
    return bass.AP(tensor=t_ap.tensor, offset=t_ap.offset,
                   ap=[list(t_ap.ap[0]), [0, C]])


def _emit(ctx, tc, nc, spt, qry, conv_w, gam, bet, mu, var, scl, out_sim):
    consts = ctx.enter_context(tc.tile_pool(name="consts", bufs=1))
    pre = ctx.enter_context(tc.tile_pool(name="pre", bufs=2))
    perq = ctx.enter_context(tc.tile_pool(name="perq", bufs=2))
    epool = ctx.enter_context(tc.tile_pool(name="epool", bufs=24))
    # PSUM: exactly 8 banks total
    ps_corr = ctx.enter_context(tc.tile_pool(name="ps_corr", bufs=6, space="PSUM"))
    ps_att = ctx.enter_context(tc.tile_pool(name="ps_att", bufs=1, space="PSUM"))
    ps_misc = ctx.enter_context(tc.tile_pool(name="ps_misc", bufs=1, space="PSUM"))

    # ---- constants ----
    ident = consts.tile([128, 128], F32)
    masks.make_identity(nc, ident[:])
    ident_h = consts.tile([128, 128], BF16)
    nc.vector.tensor_copy(ident_h[:], ident[:])
    ones128 = consts.tile([128, 1], F32)
    nc.gpsimd.memset(ones128[:], 1.0)
    ones_h = consts.tile([1, 128], BF16)         # K=1 stationary for qbar bcast
    nc.gpsimd.memset(ones_h[:], 1.0)
    oinv_rep = consts.tile([C, C], F32)          # all 1/64 -> mean-broadcast matmul
    nc.gpsimd.memset(oinv_rep[:], 1.0 / C)
    zeros = consts.tile([128, 1], F32)
    nc.gpsimd.memset(zeros[:], 0.0)
    c25e5 = consts.tile([128, 1], F32)           # bias for stats sqrt: 25*1e-5
    nc.gpsimd.memset(c25e5[:], 25.0e-5)
    c1e5 = consts.tile([128, 1], F32)            # bias for BN sqrt: 1e-5
    nc.gpsimd.memset(c1e5[:], 1.0e-5)

    # ---- input loads ----
    spt_raw = consts.tile([C, WAY * S], F32)
    nc.sync.dma_start(out=spt_raw[:].rearrange("c (w s) -> c w s", w=WAY),
                      in_=spt.rearrange("w c s -> c w s"))
    qry_raw = consts.tile([C, QPC * S], F32)
    nc.sync.dma_start(out=qry_raw[:].rearrange("c (q s) -> c q s", q=QPC),
                      in_=qry.rearrange("q c s -> c q s"))
    w_sb = consts.tile([C, C], F32)
    nc.sync.dma_start(out=w_sb[:], in_=conv_w)
    gam_sb = consts.tile([C, 1], F32)
    nc.sync.dma_start(out=gam_sb[:], in_=gam.unsqueeze(1))
    bet_sb = consts.tile([C, 1], F32)
    nc.sync.dma_start(out=bet_sb[:], in_=bet.unsqueeze(1))
    mu_sb = consts.tile([C, 1], F32)
    nc.sync.dma_start(out=mu_sb[:], in_=mu.unsqueeze(1))
    var_sb = consts.tile([C, 1], F32)
    nc.sync.dma_start(out=var_sb[:], in_=var.unsqueeze(1))
    scale_b = consts.tile([WAY, 1], F32)
    nc.gpsimd.dma_start(
        out=scale_b[:],
        in_=bass.AP(tensor=scl.tensor, offset=scl.offset, ap=[[0, WAY], [1, 1]]))

    # conv weight transposed: lhsT layout [c_in, c_out]
    wT_ps = ps_misc.tile([C, C], F32, tag="m")
    nc.tensor.transpose(wT_ps[:], w_sb[:], ident[0:C, 0:C])
    wT_sb = consts.tile([C, C], F32)
    nc.vector.tensor_copy(wT_sb[:], wT_ps[:])

    # BN fold: bns = gamma / sqrt(var + 1e-5);  bnb = beta - mean * bns
    sd = pre.tile([C, 1], F32, tag="bn")
    nc.scalar.activation(sd[:], var_sb[:], AF.Ln, bias=c1e5[0:C], scale=1.0)
    rsd = pre.tile([C, 1], F32, tag="bn")
    nc.scalar.activation(rsd[:], sd[:], AF.Exp, bias=zeros[0:C], scale=-0.5)
    bns = consts.tile([C, 1], F32)
    nc.vector.tensor_mul(bns[:], gam_sb[:], rsd[:])
    mb = pre.tile([C, 1], F32, tag="bn")
    nc.vector.tensor_mul(mb[:], mu_sb[:], bns[:])
    bnb = consts.tile([C, 1], F32)
    nc.vector.scalar_tensor_tensor(out=bnb[:], in0=mb[:], scalar=-1.0,
                                   in1=bet_sb[:], op0=OP.mult, op1=OP.add)

    # ---- channel-mean centering (normalize_feature) ----
    spt_n = consts.tile([C, WAY * S], F32)
    for w in range(WAY):
        mean_ps = ps_misc.tile([C, S], F32, tag="m")
        nc.tensor.matmul(out=mean_ps[:], lhsT=oinv_rep[:],
                         rhs=spt_raw[:, w * S:(w + 1) * S], start=True, stop=True)
        nc.vector.tensor_sub(spt_n[:, w * S:(w + 1) * S],
                             spt_raw[:, w * S:(w + 1) * S], mean_ps[:])
    qry_n = consts.tile([C, QPC * S], F32)

    def emit_qry_mean(q):
        # deferred into the pipeline: emitted right before feat(q)
        mean_ps = ps_misc.tile([C, S], F32, tag="m")
        nc.tensor.matmul(out=mean_ps[:], lhsT=oinv_rep[:],
                         rhs=qry_raw[:, q * S:(q + 1) * S], start=True, stop=True)
        nc.vector.tensor_sub(qry_n[:, q * S:(q + 1) * S],
                             qry_raw[:, q * S:(q + 1) * S], mean_ps[:])

    # ---- feature transform: conv+bn+relu, L2-normalize, centered Gram ----
    # ext layout [C, 464] bf16: [0:400 h | 400:464 centered Gram]
    def feat_transform(x_slice, ext, hT_sc):
        y_ps = ps_misc.tile([C, S], F32, tag="m")
        nc.tensor.matmul(out=y_ps[:], lhsT=wT_sb[:], rhs=x_slice,
                         start=True, stop=True)
        bnr = pre.tile([C, S], F32, tag="bnr")
        nc.scalar.activation(bnr[:], y_ps[:], AF.Relu, bias=bnb[:], scale=bns[:])
        hT_raw = pre.tile([CH, NCH * C], F32, tag="hTraw")
        nsq = pre.tile([CH, NCH], F32, tag="nsq")
        for j in range(NCH):
            tp_ps = ps_misc.tile([CH, C], F32, tag="m")
            nc.tensor.transpose(tp_ps[:], bnr[:, j * CH:(j + 1) * CH], ident[0:C, 0:C])
            nc.vector.tensor_copy(hT_raw[:, j * C:(j + 1) * C], tp_ps[:])
            sqscr = pre.tile([CH, C], F32, tag="sqscr")
            nc.vector.scalar_tensor_tensor(
                out=sqscr[:], in0=hT_raw[:, j * C:(j + 1) * C], scalar=1.0,
                in1=hT_raw[:, j * C:(j + 1) * C],
                op0=OP.mult, op1=OP.mult, accum_out=nsq[:, j:j + 1])
        nc.vector.tensor_scalar_max(nsq[:], nsq[:], 1.0e-16)
        nrm = pre.tile([CH, NCH], F32, tag="nrm")
        nc.scalar.activation(nrm[:], nsq[:], AF.Ln, bias=zeros[0:CH], scale=1.0)
        rinv = pre.tile([CH, NCH], F32, tag="rinv")
        nc.scalar.activation(rinv[:], nrm[:], AF.Exp, bias=zeros[0:CH], scale=-0.5)
        for j in range(NCH):
            nc.vector.tensor_scalar_mul(hT_sc[:, j * C:(j + 1) * C],
                                        hT_raw[:, j * C:(j + 1) * C], rinv[:, j:j + 1])
        # normalized features back to channel-major, spatial row-sum accum
        rowp = pre.tile([C, NCH], F32, tag="rowp")
        for j in range(NCH):
            bk_ps = ps_misc.tile([C, CH], BF16, tag="m")
            nc.tensor.transpose(bk_ps[:], hT_sc[:, j * C:(j + 1) * C],
                                ident_h[0:CH, 0:CH])
            nc.vector.tensor_scalar(
                out=ext[:, j * CH:(j + 1) * CH], in0=bk_ps[:],
                scalar1=1.0, scalar2=None, op0=OP.mult, op1=OP.add,
                accum_out=rowp[:, j:j + 1])
        # centered Gram: Gc = sum_p (h_p - hbar)(h_p - hbar)^T
        hbar = pre.tile([C, 1], F32, tag="hbar")
        nc.vector.tensor_reduce(out=hbar[:], in_=rowp[:], axis=AX.X, op=OP.add)
        hbT_ps = ps_misc.tile([1, C], F32, tag="m")
        nc.tensor.transpose(hbT_ps[:], hbar[:], ident[0:C, 0:C])
        hbar_h = pre.tile([1, C], BF16, tag="hbarh")
        nc.vector.tensor_scalar_mul(hbar_h[:], hbT_ps[:], 1.0 / S)
        hb_ps = ps_misc.tile([CH, C], F32, tag="m")
        nc.tensor.matmul(out=hb_ps[:], lhsT=ones_h[0:1, 0:CH], rhs=hbar_h[:],
                         start=True, stop=True)
        hTc = pre.tile([CH, NCH * C], BF16, tag="hTc")
        for j in range(NCH):
            nc.vector.tensor_sub(hTc[:, j * C:(j + 1) * C],
                                 hT_sc[:, j * C:(j + 1) * C], hb_ps[:])
        Gc_ps = ps_misc.tile([C, C], F32, tag="m")
        for j in range(NCH):
            nc.tensor.matmul(out=Gc_ps[:], lhsT=hTc[:, j * C:(j + 1) * C],
                             rhs=hTc[:, j * C:(j + 1) * C],
                             start=(j == 0), stop=(j == NCH - 1))
        nc.vector.tensor_copy(ext[:, S:S + C], Gc_ps[:])

    # ---- support features (once) ----
    s_ext = consts.tile([C, WAY * EXT], BF16)
    shT = consts.tile([CH, WAY * NCH * C], BF16)
    for w in range(WAY):
        feat_transform(spt_n[:, w * S:(w + 1) * S],
                       s_ext[:, w * EXT:(w + 1) * EXT],
                       shT[:, w * NCH * C:(w + 1) * NCH * C])

    sims = consts.tile([WAY, QPC], F32)
    nc.gpsimd.memset(sims[:], 0.0)

    # ---- per-query pipeline over 50 (query, way) pairs ----
    pairs = [(q, w) for q in range(QPC) for w in range(WAY)]
    qctx = {}      # q -> dict of per-query tiles
    st = {}        # pair index -> dict of per-pair state

    def emit_feat(q):
        emit_qry_mean(q)
        d = {}
        d["ext"] = perq.tile([C, EXT], BF16, tag="qext", name="qext")
        d["qhT"] = perq.tile([CH, NCH * C], BF16, tag="qhT", name="qhT")
        feat_transform(qry_n[:, q * S:(q + 1) * S], d["ext"][:], d["qhT"][:])
        d["spt_att"] = perq.tile([C, WAY], F32, tag="spt_att", name="spt_att")
        d["qry_att"] = perq.tile([C, WAY], F32, tag="qry_att", name="qry_att")
        qctx[q] = d

    def corr_wave(i, br):
        """4 fused corr+variance matmuls (PE) + variance math (DVE) for one
        branch.  br=0: partitions = support positions (normalize over query
        axis); br=1: partitions = query positions (normalize over support)."""
        q, w = pairs[i]
        d = qctx[q]
        if br == 0:
            lhs_all = s_ext[:, w * EXT:w * EXT + S]
            rhs = d["ext"][:]
            hT = shT[:, w * NCH * C:(w + 1) * NCH * C]
        else:
            lhs_all = d["ext"][:, 0:S]
            rhs = s_ext[:, w * EXT:(w + 1) * EXT]
            hT = d["qhT"][:]
        ps = []
        for j in range(NCH):
            cp = ps_corr.tile([CH, EXT], F32, tag="corr")
            nc.tensor.matmul(out=cp[:], lhsT=lhs_all[:, j * CH:(j + 1) * CH],
                             rhs=rhs, start=True, stop=True)
            ps.append(cp)
        # group variance*(S-1): rowsum((chunk^T Gc) o hT)
        stt = perq.tile([CH, 4], F32, tag=f"st{br}", name=f"st{br}")
        scr = perq.tile([CH, C], F32, tag=f"scr{br}", name=f"scr{br}")
        for j in range(NCH):
            nc.vector.scalar_tensor_tensor(
                out=scr[:], in0=ps[j][:, S:S + C], scalar=1.0,
                in1=hT[:, j * C:(j + 1) * C], op0=OP.mult, op1=OP.mult,
                accum_out=stt[:, j:j + 1])
        st[i][f"cp{br}"] = ps
        st[i][f"stt{br}"] = stt

    def emit_rr(i, br):
        # r = 1 / (TEMP * sqrt(var + 1e-5)),  var = sum((x-mean)^2)/399
        stt = st[i][f"stt{br}"]
        sq = perq.tile([CH, 4], F32, tag=f"sq{br}", name=f"sq{br}")
        nc.scalar.activation(sq[:], stt[:], AF.Ln, bias=c25e5[0:CH],
                             scale=(TEMP * TEMP) / (S - 1.0))
        rr = perq.tile([CH, 4], F32, tag=f"rr{br}", name=f"rr{br}")
        nc.scalar.activation(rr[:], sq[:], AF.Exp, bias=zeros[0:CH], scale=-0.5)
        st[i][f"rr{br}"] = rr

    def emit_exps(i, br):
        rr = st[i][f"rr{br}"]
        ps = st[i][f"cp{br}"]
        srowE = perq.tile([CH, 4], F32, tag=f"se{br}", name=f"se{br}")
        E = []
        for j in range(NCH):
            e = epool.tile([CH, S], BF16, tag="E", name="E")
            if br == 1 and j % 2 == 1:
                # shorten ScalarE's critical sequence: 2 of 8 denominators
                # per pair go to the vector engine instead of accum_out
                nc.scalar.activation(e[:], ps[j][:, 0:S], AF.Exp,
                                     bias=zeros[0:CH], scale=rr[:, j:j + 1])
                nc.vector.tensor_reduce(out=srowE[:, j:j + 1], in_=e[:],
                                        axis=AX.X, op=OP.add)
            else:
                nc.scalar.activation(e[:], ps[j][:, 0:S], AF.Exp,
                                     bias=zeros[0:CH], scale=rr[:, j:j + 1],
                                     accum_out=srowE[:, j:j + 1])
            E.append(e)
        st[i][f"E{br}"] = E
        st[i][f"se{br}"] = srowE

    def emit_wrec(i, br):
        wrecb = perq.tile([CH, 4], BF16, tag=f"wb{br}", name=f"wb{br}")
        with nc.allow_low_precision(reason="bf16 softmax weights are plenty"):
            nc.vector.reciprocal(wrecb[:], st[i][f"se{br}"][:])
        st[i][f"wb{br}"] = wrecb

    def emit_attn(i, br):
        q, w = pairs[i]
        d = qctx[q]
        wrecb = st[i][f"wb{br}"]
        E = st[i][f"E{br}"]
        att = ps_att.tile([C, S], F32, tag="att")
        for j in range(NCH):
            nc.tensor.matmul(out=att[:], lhsT=_bcast_col(wrecb[:, j:j + 1]),
                             rhs=E[j][:], start=(j == 0), stop=(j == NCH - 1))
        pj = perq.tile([C, S], F32, tag="pj", name="pj")
        if br == 0:
            # br 0 normalizes over the query axis -> attn_q -> pool qry_n
            nc.vector.scalar_tensor_tensor(
                out=pj[:], in0=qry_n[:, q * S:(q + 1) * S],
                scalar=1.0, in1=att[:], op0=OP.mult, op1=OP.mult,
                accum_out=d["qry_att"][:, w:w + 1])
        else:
            nc.vector.scalar_tensor_tensor(
                out=pj[:], in0=spt_n[:, w * S:(w + 1) * S],
                scalar=1.0, in1=att[:], op0=OP.mult, op1=OP.mult,
                accum_out=d["spt_att"][:, w:w + 1])

    def emit_cosine(q):
        d = qctx[q]
        P3 = perq.tile([C, 3 * WAY], F32, tag="P3", name="P3")
        nc.vector.tensor_mul(P3[:, 0:WAY], d["spt_att"][:], d["qry_att"][:])
        nc.vector.tensor_mul(P3[:, WAY:2 * WAY], d["spt_att"][:], d["spt_att"][:])
        nc.vector.tensor_mul(P3[:, 2 * WAY:3 * WAY], d["qry_att"][:], d["qry_att"][:])
        dots = ps_misc.tile([WAY, 3], F32, tag="m")
        for i in range(3):
            nc.tensor.matmul(out=dots[:, i:i + 1],
                             lhsT=P3[:, i * WAY:(i + 1) * WAY],
                             rhs=ones128[0:C], start=True, stop=True)
        nrm2 = perq.tile([WAY, 2], F32, tag="nrm2", name="nrm2")
        nc.vector.tensor_scalar_max(nrm2[:], dots[:, 1:3], 1.6e-7)
        lnn = perq.tile([WAY, 2], F32, tag="lnn", name="lnn")
        nc.scalar.activation(lnn[:], nrm2[:], AF.Ln, bias=zeros[0:WAY], scale=1.0)
        lsum = perq.tile([WAY, 1], F32, tag="lsum", name="lsum")
        nc.vector.tensor_add(lsum[:], lnn[:, 0:1], lnn[:, 1:2])
        rden = perq.tile([WAY, 1], F32, tag="rden", name="rden")
        nc.scalar.activation(rden[:], lsum[:], AF.Exp, bias=zeros[0:WAY], scale=-0.5)
        s0 = perq.tile([WAY, 1], F32, tag="s0", name="s0")
        nc.vector.tensor_mul(s0[:], dots[:, 0:1], rden[:])
        nc.vector.tensor_mul(sims[:, q:q + 1], s0[:], scale_b[:])

    # pipeline: ACT queue stays dense; PE waves flow while exps run.
    # Branch-B attention of pair i-1 is deferred until after pair i's
    # branch-B correlation wave so the B-branch variance (gating the next
    # exps) never queues behind attention matmuls on the in-order PE.
    emit_feat(0)
    st[0] = {}
    corr_wave(0, 0)
    for i, (q, w) in enumerate(pairs):
        emit_rr(i, 0)
        corr_wave(i, 1)          # PE: flows as branch-A exps free banks
        if i > 0:
            emit_wrec(i - 1, 1)
            emit_attn(i - 1, 1)
            pq, pw = pairs[i - 1]
            if pw == WAY - 1:
                emit_cosine(pq)
        emit_exps(i, 0)
        emit_rr(i, 1)
        emit_wrec(i, 0)
        emit_attn(i, 0)
        if i + 1 < len(pairs):
            st[i + 1] = {}
            corr_wave(i + 1, 0)  # PE: flows while branch-B exps run
        emit_exps(i, 1)
        if w == 2 and q + 1 < QPC:
            emit_feat(q + 1)
        st.pop(i - 2, None)
    last = len(pairs) - 1
    emit_wrec(last, 1)
    emit_attn(last, 1)
    emit_cosine(QPC - 1)

    nc.sync.dma_start(out=out_sim, in_=sims[:])


_PROGRAM = None


def _get_program():
    global _PROGRAM
    if _PROGRAM is None:
        _PROGRAM = _build_program()
    return _PROGRAM


def kernel(spt, qry, conv_w, bn_gamma, bn_beta, bn_mean, bn_var, scale):
    spt = np.ascontiguousarray(np.asarray(spt, dtype=np.float32).reshape(WAY, C, S))
    qry = np.asarray(qry, dtype=np.float32).reshape(-1, C, S)
    nq = qry.shape[0]
    npad = NCORES * QPC
    qpad = np.zeros((npad, C, S), dtype=np.float32)
    qpad[:nq] = qry
    in_maps = []
    for core in range(NCORES):
        in_maps.append({
            "spt": spt,
            "qry": np.ascontiguousarray(qpad[core * QPC:(core + 1) * QPC]),
            "conv_w": np.asarray(conv_w, dtype=np.float32),
            "bn_gamma": np.asarray(bn_gamma, dtype=np.float32),
            "bn_beta": np.asarray(bn_beta, dtype=np.float32),
            "bn_mean": np.asarray(bn_mean, dtype=np.float32),
            "bn_var": np.asarray(bn_var, dtype=np.float32),
            "scale": np.asarray(scale, dtype=np.float32),
        })
    nc = _get_program()
    trace = bool(os.environ.get("KBENCH_TRACE"))
    kw = {}
    if trace:
        import tempfile
        kw = dict(trace=True, tmpdir=tempfile.mkdtemp(prefix="ktrace_"))
    res = run_bass_kernel_spmd(nc, in_maps, list(range(NCORES)), **kw)
    if trace:
        global LAST_RESULTS
        LAST_RESULTS = res
        print("exec_time_ns:", res.exec_time_ns,
              "mean:", res.mean_exec_time_ns,
              "worst core:", res.max_exec_time_core_id)
        if res.instructions_and_trace:
            print("trace path:", res.instructions_and_trace[1])
    outs = [np.asarray(res.results[i]["out_sim"]) for i in range(NCORES)]
    full = np.concatenate([o.T for o in outs], axis=0)  # [80, WAY]
    return np.ascontiguousarray(full[:nq]).astype(np.float32)


if __name__ == "__main__":
    rng = np.random.default_rng(0)
    ins = {
        "spt": rng.standard_normal((WAY, C, 20, 20), dtype=np.float32),
        "qry": rng.standard_normal((75, C, 20, 20), dtype=np.float32),
        "conv_w": (rng.standard_normal((C, C)) * 0.1).astype(np.float32),
        "bn_gamma": np.ones(C, np.float32),
        "bn_beta": np.zeros(C, np.float32),
        "bn_mean": np.zeros(C, np.float32),
        "bn_var": np.ones(C, np.float32),
        "scale": np.ones(1, np.float32),
    }
    out = kernel(**ins)
    print(out.shape, out.dtype, out[:2])



# revision 18
# speedup vs baseline: 1.0882x; 1.0882x over previous
"""Trainium2 Bass kernel for nn_Backbone_47390669144486 (SAM-style 4D-correlation attention).

Data-parallel over the 75 queries across 8 NeuronCores (pad to 80, 10/core).
Restructured from the chunk-of-100-per-(query,way) baseline into a phased
per-query pipeline that keeps ScalarE (the bottleneck: 16M softmax exps/core)
dense and minimizes per-instruction overheads:

  * branch cq (normalize over the query axis): support positions of ALL 5
    ways are flattened to one 2000-length axis and chunked by 128 -> 16
    corr matmuls / exps per query (vs 20), full 128 ACT lanes.
  * the gauss-norm variance is computed UPFRONT from the centered-Gram
    quadratic form (var[p] = h_p^T Gc h_p) via small [*,64]/[*,320]
    matmuls + one DVE rowsum per chunk, so all 36 1/(T*sigma) factors of a
    query batch into TWO Ln/Exp activations (vs 40 small ACTs per query).
  * softmax denominators: fused accum_out on most exps (free row-sum on
    ScalarE, costs one 287ns accumulator drain each); a tunable subset is
    computed by DVE tensor_reduce instead to balance the two engines.
  * attention is deferred to a per-query tail: branch-cq uses a masked
    weight tile [128,5] (way-membership mask x 1/Z, built in ONE stride-0
    broadcast DVE op) so all 16 chunks accumulate attn_q for all 5 ways
    into a single [5,400] PSUM bank; pooling against the centered query
    features runs on the PE using a DMA-transposed [400,64] feature copy.
  * branch cs keeps per-(way, qchunk-100) layout (its softmax axis is the
    support dim of one way); attention uses the stride-0 bcast-column
    1/Z lhsT and pools on DVE as before.
  * cosine similarities for all 50 (query,way) pairs batch into one tail.

All hot-loop matmul operands are bf16; exp outputs are bf16.
"""

import os
import sys

sys.path.insert(0, "/opt/trn_rl_repo")

import numpy as np

import concourse.bass as bass
import concourse.tile as tile
from concourse import bacc, masks, mybir
from concourse.bass_utils import run_bass_kernel_spmd

F32 = mybir.dt.float32
BF16 = mybir.dt.bfloat16
AF = mybir.ActivationFunctionType
OP = mybir.AluOpType
AX = mybir.AxisListType

WAY = 5
C = 64
S = 400            # 20*20 spatial positions
SP = WAY * S       # 2000 flattened support positions
CH0 = 128          # branch-cq chunk (support axis, crosses ways)
NCH0 = (SP + CH0 - 1) // CH0   # 16 (last chunk 80)
CH1 = 100          # branch-cs chunk (query axis)
NCH1 = S // CH1    # 4
NUNITS = NCH0 + WAY * NCH1     # 36 exp units per query
NCORES = 8
QPC = 10
PAIRS = WAY * QPC  # 50
TEMP = 5.0

# how many of the 36 denominators per query go to DVE tensor_reduce
# instead of ScalarE accum_out (engine balancing).
N_DENOM_DVE = 10


def _chunk0(c):
    lo = c * CH0
    return lo, min(SP, lo + CH0) - lo  # (start, rows)


def _build_program():
    nc = bacc.Bacc("TRN2", target_bir_lowering=False, debug=False)

    spt_t = nc.dram_tensor("spt", [WAY, C, S], F32, kind="ExternalInput")
    qry_t = nc.dram_tensor("qry", [QPC, C, S], F32, kind="ExternalInput")
    w_t = nc.dram_tensor("conv_w", [C, C], F32, kind="ExternalInput")
    gam_t = nc.dram_tensor("bn_gamma", [C], F32, kind="ExternalInput")
    bet_t = nc.dram_tensor("bn_beta", [C], F32, kind="ExternalInput")
    mu_t = nc.dram_tensor("bn_mean", [C], F32, kind="ExternalInput")
    var_t = nc.dram_tensor("bn_var", [C], F32, kind="ExternalInput")
    scl_t = nc.dram_tensor("scale", [1], F32, kind="ExternalInput")
    out_t = nc.dram_tensor("out_sim", [PAIRS], F32, kind="ExternalOutput")

    from contextlib import ExitStack

    with tile.TileContext(nc) as tc, ExitStack() as ctx:
        _emit(ctx, tc, nc, spt_t.ap(), qry_t.ap(), w_t.ap(), gam_t.ap(),
              bet_t.ap(), mu_t.ap(), var_t.ap(), scl_t.ap(), out_t.ap())
    nc.compile()
    _dedup_act_table_loads(nc)
    return nc


def _dedup_act_table_loads(nc):
    """Keep one act-table load targeting natural_log_exp_and_others (serves
    Exp, Ln, Relu -- everything this kernel activates)."""
    from concourse.hw_specs import get_activation_tables

    names = list(get_activation_tables(nc.m.arch).keys())
    combined = names.index("natural_log_exp_and_others")
    kept = False
    for b in nc.m.functions[0].blocks:
        keep = []
        for i in b.instructions:
            if type(i).__name__ == "InstLoadActFuncSet":
                si = i.sync_info
                assert si is None or (not si.on_wait and not si.on_update)
                if kept:
                    continue
                i.act_func_set_id = combined
                kept = True
            keep.append(i)
        if len(keep) != len(b.instructions):
            b.instructions[:] = keep


def _ap3(t_ap, dims):
    """Build an AP over tile t_ap with explicit free dims [(stride, n), ...]."""
    return bass.AP(tensor=t_ap.tensor, offset=t_ap.offset,
                   ap=[list(t_ap.ap[0])] + [list(d) for d in dims])


def _bcast_col(t_ap, n):
    """[P,1] AP -> [P,n] stride-0 free-dim broadcast."""
    return bass.AP(tensor=t_ap.tensor, offset=t_ap.offset,
                   ap=[list(t_ap.ap[0]), [0, n]])


def _emit(ctx, tc, nc, spt, qry, conv_w, gam, bet, mu, var, scl, out_sim):
    consts = ctx.enter_context(tc.tile_pool(name="consts", bufs=1))
    pre = ctx.enter_context(tc.tile_pool(name="pre", bufs=2))
    perq = ctx.enter_context(tc.tile_pool(name="perq", bufs=2))
    e0pool = ctx.enter_context(tc.tile_pool(name="e0", bufs=2 * NCH0))
    e1pool = ctx.enter_context(tc.tile_pool(name="e1", bufs=2 * WAY * NCH1))
    # PSUM: 8 banks total
    ps_corr = ctx.enter_context(tc.tile_pool(name="ps_corr", bufs=4, space="PSUM"))
    ps_attq = ctx.enter_context(tc.tile_pool(name="ps_attq", bufs=1, space="PSUM"))
    ps_atts = ctx.enter_context(tc.tile_pool(name="ps_atts", bufs=1, space="PSUM"))
    ps_misc = ctx.enter_context(tc.tile_pool(name="ps_misc", bufs=2, space="PSUM"))

    # ---- constants ----
    ident = consts.tile([128, 128], F32)
    masks.make_identity(nc, ident[:])
    ident_h = consts.tile([128, 128], BF16)
    nc.vector.tensor_copy(ident_h[:], ident[:])
    ones128 = consts.tile([128, 1], F32)
    nc.gpsimd.memset(ones128[:], 1.0)
    ones_h = consts.tile([1, 128], BF16)
    nc.gpsimd.memset(ones_h[:], 1.0)
    oinv_rep = consts.tile([C, C], F32)          # all 1/64 -> channel-mean matmul
    nc.gpsimd.memset(oinv_rep[:], 1.0 / C)
    zeros = consts.tile([128, 1], F32)
    nc.gpsimd.memset(zeros[:], 0.0)
    c25e5 = consts.tile([128, 1], F32)           # bias for stats sqrt: 25*1e-5
    nc.gpsimd.memset(c25e5[:], 25.0e-5)
    c1e5 = consts.tile([128, 1], F32)            # bias for BN sqrt: 1e-5
    nc.gpsimd.memset(c1e5[:], 1.0e-5)

    # way-membership masks for the cq-branch attention: mask_all[p, c*WAY+w]=1
    # iff global support position c*128+p belongs to way w.
    mask_all = consts.tile([128, NCH0 * WAY], BF16)
    nc.gpsimd.memset(mask_all[:], 0.0)
    ones_col_h = consts.tile([128, 1], BF16)
    nc.gpsimd.memset(ones_col_h[:], 1.0)
    for cix in range(NCH0):
        lo, rows = _chunk0(cix)
        r = 0
        while r < rows:
            w = (lo + r) // S
            seg = min(rows - r, (w + 1) * S - (lo + r))
            # mid-partition writes need DMA (engines can't start at p>0 here)
            nc.sync.dma_start(
                out=mask_all[r:r + seg, cix * WAY + w:cix * WAY + w + 1],
                in_=ones_col_h[0:seg])
            r += seg

    # ---- input loads ----
    spt_raw = consts.tile([C, SP], F32)
    nc.sync.dma_start(out=spt_raw[:].rearrange("c (w s) -> c w s", w=WAY),
                      in_=spt.rearrange("w c s -> c w s"))
    qry_raw = consts.tile([C, QPC * S], F32)
    nc.sync.dma_start(out=qry_raw[:].rearrange("c (q s) -> c q s", q=QPC),
                      in_=qry.rearrange("q c s -> c q s"))
    w_sb = consts.tile([C, C], F32)
    nc.sync.dma_start(out=w_sb[:], in_=conv_w)
    gam_sb = consts.tile([C, 1], F32)
    nc.sync.dma_start(out=gam_sb[:], in_=gam.unsqueeze(1))
    bet_sb = consts.tile([C, 1], F32)
    nc.sync.dma_start(out=bet_sb[:], in_=bet.unsqueeze(1))
    mu_sb = consts.tile([C, 1], F32)
    nc.sync.dma_start(out=mu_sb[:], in_=mu.unsqueeze(1))
    var_sb = consts.tile([C, 1], F32)
    nc.sync.dma_start(out=var_sb[:], in_=var.unsqueeze(1))
    scale_b = consts.tile([PAIRS, 1], F32)
    nc.gpsimd.dma_start(
        out=scale_b[:],
        in_=bass.AP(tensor=scl.tensor, offset=scl.offset, ap=[[0, PAIRS], [1, 1]]))

    # conv weight transposed: lhsT layout [c_in, c_out]
    wT_ps = ps_misc.tile([C, C], F32, tag="m")
    nc.tensor.transpose(wT_ps[:], w_sb[:], ident[0:C, 0:C])
    wT_sb = consts.tile([C, C], F32)
    nc.vector.tensor_copy(wT_sb[:], wT_ps[:])

    # BN fold: bns = gamma / sqrt(var + 1e-5);  bnb = beta - mean * bns
    sd = pre.tile([C, 1], F32, tag="bn")
    nc.scalar.activation(sd[:], var_sb[:], AF.Ln, bias=c1e5[0:C], scale=1.0)
    rsd = pre.tile([C, 1], F32, tag="bn")
    nc.scalar.activation(rsd[:], sd[:], AF.Exp, bias=zeros[0:C], scale=-0.5)
    bns = consts.tile([C, 1], F32)
    nc.vector.tensor_mul(bns[:], gam_sb[:], rsd[:])
    mb = pre.tile([C, 1], F32, tag="bn")
    nc.vector.tensor_mul(mb[:], mu_sb[:], bns[:])
    bnb = consts.tile([C, 1], F32)
    nc.vector.scalar_tensor_tensor(out=bnb[:], in0=mb[:], scalar=-1.0,
                                   in1=bet_sb[:], op0=OP.mult, op1=OP.add)

    # fold the channel-mean centering into the conv weight (rank-1 update):
    # y = W(x - xbar) = W'x with W' = W - wbar 1^T, wbar = W @ ones/64.
    # Keeps the hot loop free of fp32 matmuls (fp32 PE ops hold the clock
    # gate at 1.2 GHz).
    wbar_ps = ps_misc.tile([1, C], F32, tag="m")
    nc.tensor.matmul(out=wbar_ps[:], lhsT=oinv_rep[:, 0:1], rhs=wT_sb[:],
                     start=True, stop=True)
    wbarT = consts.tile([1, C], BF16)
    nc.vector.tensor_copy(wbarT[:], wbar_ps[:])
    wrep_ps = ps_misc.tile([C, C], F32, tag="m")
    nc.tensor.matmul(out=wrep_ps[:], lhsT=ones_h[0:1, 0:C], rhs=wbarT[:],
                     start=True, stop=True)
    wc_h = consts.tile([C, C], BF16)
    nc.vector.tensor_sub(wc_h[:], wT_sb[:], wrep_ps[:])

    # ---- channel-mean centering of the support (for cs-branch pooling) ----
    spt_n = consts.tile([C, SP], F32)
    for w in range(WAY):
        mean_ps = ps_misc.tile([C, S], F32, tag="m")
        nc.tensor.matmul(out=mean_ps[:], lhsT=oinv_rep[:],
                         rhs=spt_raw[:, w * S:(w + 1) * S], start=True, stop=True)
        nc.vector.tensor_sub(spt_n[:, w * S:(w + 1) * S],
                             spt_raw[:, w * S:(w + 1) * S], mean_ps[:])
    # raw support in bf16 (feature-transform input; centering is in W')
    sptb = consts.tile([C, SP], BF16)
    nc.vector.tensor_copy(sptb[:], spt_raw[:])

    # ---- feature transform: conv+bn+relu, L2-normalize over channels, Gram ----
    def feat_transform(x_slice, h_out, gc_out, hT_out):
        """x_slice: [C, S] bf16 RAW features (centering folded into wc_h);
        h_out: [C, S] bf16 AP for the normalized features (channel-major);
        gc_out: [C, C] bf16 AP for the centered Gram; hT_out: [CH1, NCH1*C]
        bf16 tile (position-major, 100-chunks) or None."""
        y_ps = ps_misc.tile([C, S], F32, tag="m")
        nc.tensor.matmul(out=y_ps[:], lhsT=wc_h[:], rhs=x_slice,
                         start=True, stop=True)
        bnr = pre.tile([C, S], BF16, tag="bnr")
        nc.scalar.activation(bnr[:], y_ps[:], AF.Relu, bias=bnb[:], scale=bns[:])
        hT_raw = pre.tile([CH1, NCH1 * C], BF16, tag="hTraw")
        nsq = pre.tile([CH1, NCH1], F32, tag="nsq")
        for j in range(NCH1):
            tp_ps = ps_misc.tile([CH1, C], BF16, tag="m")
            nc.tensor.transpose(tp_ps[:], bnr[:, j * CH1:(j + 1) * CH1],
                                ident_h[0:C, 0:C])
            nc.vector.tensor_copy(hT_raw[:, j * C:(j + 1) * C], tp_ps[:])
            sqscr = pre.tile([CH1, C], F32, tag="sqscr")
            nc.vector.scalar_tensor_tensor(
                out=sqscr[:], in0=hT_raw[:, j * C:(j + 1) * C], scalar=1.0,
                in1=hT_raw[:, j * C:(j + 1) * C],
                op0=OP.mult, op1=OP.mult, accum_out=nsq[:, j:j + 1])
        nc.vector.tensor_scalar_max(nsq[:], nsq[:], 1.0e-16)
        nrm = pre.tile([CH1, NCH1], F32, tag="nrm")
        nc.scalar.activation(nrm[:], nsq[:], AF.Ln, bias=zeros[0:CH1], scale=1.0)
        rinv = pre.tile([CH1, NCH1], F32, tag="rinv")
        nc.scalar.activation(rinv[:], nrm[:], AF.Exp, bias=zeros[0:CH1], scale=-0.5)
        hT_sc = hT_out if hT_out is not None else pre.tile(
            [CH1, NCH1 * C], BF16, tag="hTsc")
        for j in range(NCH1):
            nc.vector.tensor_scalar_mul(hT_sc[:, j * C:(j + 1) * C],
                                        hT_raw[:, j * C:(j + 1) * C], rinv[:, j:j + 1])
        # normalized features back to channel-major + spatial row-sum accum
        rowp = pre.tile([C, NCH1], F32, tag="rowp")
        for j in range(NCH1):
            bk_ps = ps_misc.tile([C, CH1], BF16, tag="m")
            nc.tensor.transpose(bk_ps[:], hT_sc[:, j * C:(j + 1) * C],
                                ident_h[0:CH1, 0:CH1])
            nc.vector.tensor_scalar(
                out=h_out[:, j * CH1:(j + 1) * CH1], in0=bk_ps[:],
                scalar1=1.0, scalar2=None, op0=OP.mult, op1=OP.add,
                accum_out=rowp[:, j:j + 1])
        # centered Gram: Gc = sum_p (h_p - hbar)(h_p - hbar)^T
        hbar = pre.tile([C, 1], F32, tag="hbar")
        nc.vector.tensor_reduce(out=hbar[:], in_=rowp[:], axis=AX.X, op=OP.add)
        hbar_b = pre.tile([C, 1], BF16, tag="hbarb")
        nc.vector.tensor_copy(hbar_b[:], hbar[:])
        hbT_ps = ps_misc.tile([1, C], BF16, tag="m")
        nc.tensor.transpose(hbT_ps[:], hbar_b[:], ident_h[0:C, 0:C])
        hbar_h = pre.tile([1, C], BF16, tag="hbarh")
        nc.vector.tensor_scalar_mul(hbar_h[:], hbT_ps[:], 1.0 / S)
        hb_ps = ps_misc.tile([CH1, C], F32, tag="m")
        nc.tensor.matmul(out=hb_ps[:], lhsT=ones_h[0:1, 0:CH1], rhs=hbar_h[:],
                         start=True, stop=True)
        hTc = pre.tile([CH1, NCH1 * C], BF16, tag="hTc")
        for j in range(NCH1):
            nc.vector.tensor_sub(hTc[:, j * C:(j + 1) * C],
                                 hT_sc[:, j * C:(j + 1) * C], hb_ps[:])
        Gc_ps = ps_misc.tile([C, C], F32, tag="m")
        for j in range(NCH1):
            nc.tensor.matmul(out=Gc_ps[:], lhsT=hTc[:, j * C:(j + 1) * C],
                             rhs=hTc[:, j * C:(j + 1) * C],
                             start=(j == 0), stop=(j == NCH1 - 1))
        nc.vector.tensor_copy(gc_out, Gc_ps[:])

    # ---- support features (once) ----
    s_all = consts.tile([C, SP], BF16)            # normalized feats, all ways
    gc_s5 = consts.tile([C, WAY * C], BF16)       # 5 centered Grams
    for w in range(WAY):
        feat_transform(sptb[:, w * S:(w + 1) * S],
                       s_all[:, w * S:(w + 1) * S],
                       gc_s5[:, w * C:(w + 1) * C], None)

    # position-major support features, 128-chunks across ways (for cq stt)
    hT_all = consts.tile([128, NCH0 * C], BF16)
    for cix in range(NCH0):
        lo, rows = _chunk0(cix)
        tp = ps_misc.tile([128, C], BF16, tag="m")
        nc.tensor.transpose(tp[0:rows], s_all[:, lo:lo + rows], ident_h[0:C, 0:C])
        nc.vector.tensor_copy(hT_all[0:rows, cix * C:(cix + 1) * C], tp[0:rows])

    # attention/pooling accumulators over all pairs
    sall = consts.tile([C, PAIRS], F32)   # spt_att columns (pair-major q*5+w)
    qall = consts.tile([C, PAIRS], F32)   # qry_att columns
    attqT = consts.tile([128, 4 * WAY], BF16)     # attn_q^T chunks (per query)
    nc.gpsimd.memset(attqT[:], 0.0)

    # per-query persistent feature arrays (all 10 queries computed upfront,
    # interleaved with the 5 support transforms so every engine stays busy
    # during the startup phase and the hot loop stays pure corr->exp->attn)
    qh_all = consts.tile([C, QPC * S], BF16)
    gcq_all = consts.tile([C, QPC * C], BF16)
    qhT_all = consts.tile([CH1, QPC * NCH1 * C], BF16)
    qnT_all = consts.tile([128, QPC * 4 * C], BF16)

    # ---------------- per-query pipeline ----------------
    qctx = {}

    def emit_feat(q):
        d = {}
        # raw bf16 features, 512-padded for the DMA transpose
        qrb = pre.tile([C, 512], BF16, tag="qrb", name="qrb")
        nc.gpsimd.memset(qrb[:, S:512], 0.0)
        nc.vector.tensor_copy(qrb[:, 0:S], qry_raw[:, q * S:(q + 1) * S])
        d["qh"] = qh_all[:, q * S:(q + 1) * S]
        d["gcq"] = gcq_all[:, q * C:(q + 1) * C]
        d["qhT"] = qhT_all[:, q * NCH1 * C:(q + 1) * NCH1 * C]
        feat_transform(qrb[:, 0:S], d["qh"], d["gcq"], d["qhT"])
        # position-major raw features via DMA transpose, then channel-mean
        # centering per position (row) on DVE: qnT = qnT_raw - rowmean.
        qnTr = pre.tile([128, 4 * C], BF16, tag="qnTr", name="qnTr")
        for j in range(4):
            nc.sync.dma_start_transpose(qnTr[:, j * C:(j + 1) * C],
                                        qrb[:, j * 128:(j + 1) * 128])
        d["qnT"] = qnT_all[:, q * 4 * C:(q + 1) * 4 * C]
        qmean = pre.tile([128, 4], F32, tag="qmean", name="qmean")
        for j in range(4):
            nc.vector.tensor_reduce(out=qmean[:, j:j + 1],
                                    in_=qnTr[:, j * C:(j + 1) * C],
                                    axis=AX.X, op=OP.add)
        nc.vector.tensor_scalar_mul(qmean[:], qmean[:], 1.0 / C)
        for j in range(4):
            nc.vector.tensor_scalar(
                out=d["qnT"][:, j * C:(j + 1) * C],
                in0=qnTr[:, j * C:(j + 1) * C],
                scalar1=qmean[:, j:j + 1], scalar2=None,
                op0=OP.subtract, op1=OP.bypass)
        qctx[q] = d

    def emit_uvar(q):
        """All 36 variance quadratic forms + batched rr for query q.
        stt layout: [128, NCH0 + WAY*NCH1]; cols 0:16 = cq chunks (rows =
        chunk rows), cols 16:36 = cs (way,chunk) (rows 0:100)."""
        d = qctx[q]
        stt = perq.tile([128, NUNITS], F32, tag="stt", name="stt")
        nc.gpsimd.memset(stt[:], 0.0)
        for cix in range(NCH0):
            lo, rows = _chunk0(cix)
            u_ps = ps_misc.tile([128, C], F32, tag="m")
            nc.tensor.matmul(out=u_ps[0:rows], lhsT=s_all[:, lo:lo + rows],
                             rhs=d["gcq"], start=True, stop=True)
            scr = pre.tile([128, C], F32, tag="uscr")
            nc.vector.scalar_tensor_tensor(
                out=scr[0:rows], in0=u_ps[0:rows], scalar=1.0,
                in1=hT_all[0:rows, cix * C:(cix + 1) * C],
                op0=OP.mult, op1=OP.mult, accum_out=stt[0:rows, cix:cix + 1])
        qhTq = d["qhT"]
        for j in range(NCH1):
            u5_ps = ps_misc.tile([CH1, WAY * C], F32, tag="m")
            nc.tensor.matmul(out=u5_ps[:],
                             lhsT=d["qh"][:, j * CH1:(j + 1) * CH1],
                             rhs=gc_s5[:], start=True, stop=True)
            for w in range(WAY):
                scr = pre.tile([CH1, C], F32, tag="uscr")
                col = NCH0 + j * WAY + w
                nc.vector.scalar_tensor_tensor(
                    out=scr[:], in0=u5_ps[:, w * C:(w + 1) * C], scalar=1.0,
                    in1=qhTq[:, j * C:(j + 1) * C],
                    op0=OP.mult, op1=OP.mult, accum_out=stt[0:CH1, col:col + 1])
        # rr = 1/(TEMP*sqrt(var+1e-5)), var = stt/399; junk rows stay finite.
        sq = perq.tile([128, NUNITS], F32, tag="sq", name="sq")
        nc.scalar.activation(sq[:], stt[:], AF.Ln, bias=c25e5[:],
                             scale=(TEMP * TEMP) / (S - 1.0))
        rr = perq.tile([128, NUNITS], F32, tag="rr", name="rr")
        nc.scalar.activation(rr[:], sq[:], AF.Exp, bias=zeros[:], scale=-0.5)
        d["rr"] = rr
        d["z"] = perq.tile([128, NUNITS], F32, tag="z", name="z")
        d["E0"] = [None] * NCH0
        d["E1"] = [None] * (WAY * NCH1)

    # unit list: interleave cq chunks and cs (way, chunk) units
    units = []
    u0 = [("cq", cix) for cix in range(NCH0)]
    u1 = [("cs", w * NCH1 + k) for w in range(WAY) for k in range(NCH1)]
    i0 = i1 = 0
    for i in range(NUNITS):
        # ratio 16:20 -> alternate with slight cs surplus
        if (i * NCH0) // NUNITS >= i0 + (1 if i1 > i0 else 0) and i0 < NCH0:
            units.append(u0[i0]); i0 += 1
        elif i1 < len(u1):
            units.append(u1[i1]); i1 += 1
        else:
            units.append(u0[i0]); i0 += 1

    def emit_unit(q, ui):
        """One corr matmul + exp (+ denominator) unit."""
        d = qctx[q]
        kind, ix = units[ui]
        on_dve = ui < N_DENOM_DVE  # spread: first units' denoms on DVE
        if kind == "cq":
            cix = ix
            lo, rows = _chunk0(cix)
            cp = ps_corr.tile([128, S], F32, tag="corr")
            nc.tensor.matmul(out=cp[0:rows], lhsT=s_all[:, lo:lo + rows],
                             rhs=d["qh"][:], start=True, stop=True)
            e = e0pool.tile([128, S], BF16, tag="E0", name="E0")
            col = cix
            zcol = d["z"][0:rows, col:col + 1]
            if on_dve:
                nc.scalar.activation(e[0:rows], cp[0:rows], AF.Exp,
                                     bias=zeros[0:rows],
                                     scale=d["rr"][0:rows, col:col + 1])
                nc.vector.tensor_reduce(out=zcol, in_=e[0:rows], axis=AX.X,
                                        op=OP.add)
            else:
                nc.scalar.activation(e[0:rows], cp[0:rows], AF.Exp,
                                     bias=zeros[0:rows],
                                     scale=d["rr"][0:rows, col:col + 1],
                                     accum_out=zcol)
            d["E0"][cix] = e
        else:
            w, k = ix // NCH1, ix % NCH1
            cp = ps_corr.tile([128, S], F32, tag="corr")
            nc.tensor.matmul(out=cp[0:CH1],
                             lhsT=d["qh"][:, k * CH1:(k + 1) * CH1],
                             rhs=s_all[:, w * S:(w + 1) * S], start=True, stop=True)
            e = e1pool.tile([CH1, S], BF16, tag="E1", name="E1")
            col = NCH0 + k * WAY + w
            zcol = d["z"][0:CH1, col:col + 1]
            if on_dve:
                nc.scalar.activation(e[:], cp[0:CH1], AF.Exp, bias=zeros[0:CH1],
                                     scale=d["rr"][0:CH1, col:col + 1])
                nc.vector.tensor_reduce(out=zcol, in_=e[:], axis=AX.X, op=OP.add)
            else:
                nc.scalar.activation(e[:], cp[0:CH1], AF.Exp, bias=zeros[0:CH1],
                                     scale=d["rr"][0:CH1, col:col + 1],
                                     accum_out=zcol)
            d["E1"][ix] = e

    def tail_ops(q):
        """Attention + pooling for query q as a list of thunks (emitted
        interleaved into the next query's exp stream)."""
        d = qctx[q]
        ops = []

        def recips():
            wrec = perq.tile([128, NUNITS], BF16, tag="wrec", name="wrec")
            with nc.allow_low_precision(reason="bf16 softmax weights"):
                nc.vector.reciprocal(wrec[:], d["z"][:])
            d["wrec"] = wrec
            # masked cq attention weights in ONE stride-0 bcast op:
            # mw[p, c, w] = mask_all[p, c, w] * wrec[p, c]
            mw = perq.tile([128, NCH0 * WAY], BF16, tag="mw", name="mw")
            nc.vector.tensor_mul(
                _ap3(mw[:], [[WAY, NCH0], [1, WAY]]),
                _ap3(mask_all[:], [[WAY, NCH0], [1, WAY]]),
                _ap3(wrec[:], [[1, NCH0], [0, WAY]]))
            d["mw"] = mw
        ops.append(recips)

        # cq attention: 16 masked matmuls accumulate attn_q [5, 400]
        attq = {}

        def mk_cq_attn(cix, first, last):
            def f():
                if first:
                    attq["t"] = ps_attq.tile([WAY, S], F32, tag="attq", name="attq")
                lo, rows = _chunk0(cix)
                nc.tensor.matmul(
                    out=attq["t"][:],
                    lhsT=d["mw"][0:rows, cix * WAY:(cix + 1) * WAY],
                    rhs=d["E0"][cix][0:rows], start=first, stop=last)
            return f
        for cix in range(NCH0):
            ops.append(mk_cq_attn(cix, cix == 0, cix == NCH0 - 1))

        def attq_tail():
            # attn_q -> SBUF bf16, transpose 128-pieces, pool via PE
            aq = perq.tile([WAY, S], BF16, tag="aq", name="aq")
            nc.vector.tensor_copy(aq[:], attq["t"][:])
            d["aq"] = aq
        ops.append(attq_tail)

        def mk_attq_tp(j):
            def f():
                npc = min(128, S - j * 128)
                tp = ps_misc.tile([128, WAY], BF16, tag="m")
                nc.tensor.transpose(tp[0:npc], d["aq"][:, j * 128:j * 128 + npc],
                                    ident_h[0:WAY, 0:WAY])
                nc.vector.tensor_copy(attqT[0:npc, j * WAY:(j + 1) * WAY],
                                      tp[0:npc])
            return f
        for j in range(4):
            ops.append(mk_attq_tp(j))

        qatt = {}

        def mk_qpool(j, first, last):
            def f():
                if first:
                    qatt["t"] = ps_misc.tile([C, WAY], F32, tag="m", name="qatt")
                npc = min(128, S - j * 128)
                nc.tensor.matmul(out=qatt["t"][:],
                                 lhsT=qctx[q]["qnT"][0:npc, j * C:(j + 1) * C],
                                 rhs=attqT[0:npc, j * WAY:(j + 1) * WAY],
                                 start=first, stop=last)
                if last:
                    nc.vector.tensor_copy(qall[:, q * WAY:(q + 1) * WAY],
                                          qatt["t"][:])
            return f
        for j in range(4):
            ops.append(mk_qpool(j, j == 0, j == 3))

        # cs attention per way: bcast-column 1/Z lhsT, pool on DVE
        atts = {}

        def mk_cs_attn(w, k, first, last):
            def f():
                if first:
                    atts["t"] = ps_atts.tile([C, S], F32, tag="atts", name="atts")
                col = NCH0 + k * WAY + w
                nc.tensor.matmul(
                    out=atts["t"][:],
                    lhsT=_bcast_col(d["wrec"][0:CH1, col:col + 1], C),
                    rhs=d["E1"][w * NCH1 + k][:], start=first, stop=last)
                if last:
                    pj = pre.tile([C, S], F32, tag="pj")
                    nc.vector.scalar_tensor_tensor(
                        out=pj[:], in0=spt_n[:, w * S:(w + 1) * S],
                        scalar=1.0, in1=atts["t"][:], op0=OP.mult, op1=OP.mult,
                        accum_out=sall[:, q * WAY + w:q * WAY + w + 1])
            return f
        for w in range(WAY):
            for k in range(NCH1):
                ops.append(mk_cs_attn(w, k, k == 0, k == NCH1 - 1))

        def cleanup():
            qctx.pop(q, None)
        ops.append(cleanup)
        return ops

    def emit_cosine_all():
        p3 = pre.tile([C, 3 * PAIRS], F32, tag="p3")
        nc.vector.tensor_mul(p3[:, 0:PAIRS], sall[:], qall[:])
        nc.vector.tensor_mul(p3[:, PAIRS:2 * PAIRS], sall[:], sall[:])
        nc.vector.tensor_mul(p3[:, 2 * PAIRS:3 * PAIRS], qall[:], qall[:])
        dots = ps_misc.tile([PAIRS, 3], F32, tag="m")
        for i in range(3):
            nc.tensor.matmul(out=dots[:, i:i + 1],
                             lhsT=p3[:, i * PAIRS:(i + 1) * PAIRS],
                             rhs=ones128[0:C], start=True, stop=True)
        nrm2 = pre.tile([PAIRS, 2], F32, tag="nrm2")
        nc.vector.tensor_scalar_max(nrm2[:], dots[:, 1:3], 1.6e-7)
        lnn = pre.tile([PAIRS, 2], F32, tag="lnn")
        nc.scalar.activation(lnn[:], nrm2[:], AF.Ln, bias=zeros[0:PAIRS], scale=1.0)
        lsum = pre.tile([PAIRS, 1], F32, tag="lsum")
        nc.vector.tensor_add(lsum[:], lnn[:, 0:1], lnn[:, 1:2])
        rden = pre.tile([PAIRS, 1], F32, tag="rden")
        nc.scalar.activation(rden[:], lsum[:], AF.Exp, bias=zeros[0:PAIRS], scale=-0.5)
        s0 = pre.tile([PAIRS, 1], F32, tag="s0")
        nc.vector.tensor_mul(s0[:], dots[:, 0:1], rden[:])
        sims = pre.tile([PAIRS, 1], F32, tag="sims")
        nc.vector.tensor_mul(sims[:], s0[:], scale_b[:])
        nc.sync.dma_start(out=out_sim.unsqueeze(1), in_=sims[:])

    # ---------------- top-level schedule ----------------
    emit_feat(0)
    emit_feat(1)
    emit_uvar(0)
    pending = []           # tail thunks of the previous query
    for q in range(QPC):
        for ui in range(NUNITS):
            emit_unit(q, ui)
            # spread previous query's attention/pool tail into this stream
            take = (len(pending) + NUNITS - 1 - ui) // (NUNITS - ui)
            for _ in range(take):
                pending.pop(0)()
            # next-query prep at fixed points
            if q + 2 < QPC and ui == 8:
                emit_feat(q + 2)
            if q + 1 < QPC and ui == 22:
                emit_uvar(q + 1)
        pending = tail_ops(q)
    for f in pending:
        f()
    emit_cosine_all()


_PROGRAM = None


def _get_program():
    global _PROGRAM
    if _PROGRAM is None:
        _PROGRAM = _build_program()
    return _PROGRAM


def kernel(spt, qry, conv_w, bn_gamma, bn_beta, bn_mean, bn_var, scale):
    spt = np.ascontiguousarray(np.asarray(spt, dtype=np.float32).reshape(WAY, C, S))
    qry = np.asarray(qry, dtype=np.float32).reshape(-1, C, S)
    nq = qry.shape[0]
    npad = NCORES * QPC
    qpad = np.zeros((npad, C, S), dtype=np.float32)
    qpad[:nq] = qry
    in_maps = []
    for core in range(NCORES):
        in_maps.append({
            "spt": spt,
            "qry": np.ascontiguousarray(qpad[core * QPC:(core + 1) * QPC]),
            "conv_w": np.asarray(conv_w, dtype=np.float32),
            "bn_gamma": np.asarray(bn_gamma, dtype=np.float32),
            "bn_beta": np.asarray(bn_beta, dtype=np.float32),
            "bn_mean": np.asarray(bn_mean, dtype=np.float32),
            "bn_var": np.asarray(bn_var, dtype=np.float32),
            "scale": np.asarray(scale, dtype=np.float32),
        })
    nc = _get_program()
    trace = bool(os.environ.get("KBENCH_TRACE"))
    kw = {}
    if trace:
        import tempfile
        kw = dict(trace=True, tmpdir=tempfile.mkdtemp(prefix="ktrace_"))
    res = run_bass_kernel_spmd(nc, in_maps, list(range(NCORES)), **kw)
    if trace:
        global LAST_RESULTS
        LAST_RESULTS = res
        print("exec_time_ns:", res.exec_time_ns,
              "mean:", res.mean_exec_time_ns,
              "worst core:", res.max_exec_time_core_id)
        if res.instructions_and_trace:
            print("trace path:", res.instructions_and_trace[1])
    outs = [np.asarray(res.results[i]["out_sim"]) for i in range(NCORES)]
    full = np.concatenate([o.reshape(QPC, WAY) for o in outs], axis=0)  # [80, 5]
    return np.ascontiguousarray(full[:nq]).astype(np.float32)


if __name__ == "__main__":
    rng = np.random.default_rng(0)
    ins = {
        "spt": rng.standard_normal((WAY, C, 20, 20), dtype=np.float32),
        "qry": rng.standard_normal((75, C, 20, 20), dtype=np.float32),
        "conv_w": (rng.standard_normal((C, C)) * 0.1).astype(np.float32),
        "bn_gamma": np.ones(C, np.float32),
        "bn_beta": np.zeros(C, np.float32),
        "bn_mean": np.zeros(C, np.float32),
        "bn_var": np.ones(C, np.float32),
        "scale": np.ones(1, np.float32),
    }
    out = kernel(**ins)
    print(out.shape, out.dtype, out[:2])


# revision 25
# speedup vs baseline: 1.1442x; 1.0515x over previous
"""Trainium2 Bass kernel for nn_Backbone_47390669144486 (SAM-style 4D-correlation attention).

Data-parallel over the 75 queries across 8 NeuronCores (pad to 80, 10/core).
Restructured from the chunk-of-100-per-(query,way) baseline into a phased
per-query pipeline that keeps ScalarE (the bottleneck: 16M softmax exps/core)
dense and minimizes per-instruction overheads:

  * branch cq (normalize over the query axis): support positions of ALL 5
    ways are flattened to one 2000-length axis and chunked by 128 -> 16
    corr matmuls / exps per query (vs 20), full 128 ACT lanes.
  * the gauss-norm variance is computed UPFRONT from the centered-Gram
    quadratic form (var[p] = h_p^T Gc h_p) via small [*,64]/[*,320]
    matmuls + one DVE rowsum per chunk, so all 36 1/(T*sigma) factors of a
    query batch into TWO Ln/Exp activations (vs 40 small ACTs per query).
  * softmax denominators: fused accum_out on most exps (free row-sum on
    ScalarE, costs one 287ns accumulator drain each); a tunable subset is
    computed by DVE tensor_reduce instead to balance the two engines.
  * attention is deferred to a per-query tail: branch-cq uses a masked
    weight tile [128,5] (way-membership mask x 1/Z, built in ONE stride-0
    broadcast DVE op) so all 16 chunks accumulate attn_q for all 5 ways
    into a single [5,400] PSUM bank; pooling against the centered query
    features runs on the PE using a DMA-transposed [400,64] feature copy.
  * branch cs keeps per-(way, qchunk-100) layout (its softmax axis is the
    support dim of one way); attention uses the stride-0 bcast-column
    1/Z lhsT and pools on DVE as before.
  * cosine similarities for all 50 (query,way) pairs batch into one tail.

All hot-loop matmul operands are bf16; exp outputs are bf16.
"""

import os
import sys

sys.path.insert(0, "/opt/trn_rl_repo")

import numpy as np

import concourse.bass as bass
import concourse.tile as tile
from concourse import bacc, masks, mybir
from concourse.bass_utils import run_bass_kernel_spmd

F32 = mybir.dt.float32
BF16 = mybir.dt.bfloat16
AF = mybir.ActivationFunctionType
OP = mybir.AluOpType
AX = mybir.AxisListType

WAY = 5
C = 64
S = 400            # 20*20 spatial positions
SP = WAY * S       # 2000 flattened support positions
CH0 = 128          # branch-cq chunk (support axis, crosses ways)
NCH0 = (SP + CH0 - 1) // CH0   # 16 (last chunk 80)
CH1 = 100          # branch-cs chunk (query axis)
NCH1 = S // CH1    # 4
NUNITS = NCH0 + WAY * NCH1     # 36 exp units per query
NCORES = 8
QPC = 10
PAIRS = WAY * QPC  # 50
TEMP = 5.0

# how many of the 36 denominators per query go to DVE tensor_reduce
# instead of ScalarE accum_out (engine balancing).
N_DENOM_DVE = 11


def _chunk0(c):
    lo = c * CH0
    return lo, min(SP, lo + CH0) - lo  # (start, rows)


def _build_program():
    nc = bacc.Bacc("TRN2", target_bir_lowering=False, debug=False)

    spt_t = nc.dram_tensor("spt", [WAY, C, S], F32, kind="ExternalInput")
    qry_t = nc.dram_tensor("qry", [QPC, C, S], F32, kind="ExternalInput")
    w_t = nc.dram_tensor("conv_w", [C, C], F32, kind="ExternalInput")
    gam_t = nc.dram_tensor("bn_gamma", [C], F32, kind="ExternalInput")
    bet_t = nc.dram_tensor("bn_beta", [C], F32, kind="ExternalInput")
    mu_t = nc.dram_tensor("bn_mean", [C], F32, kind="ExternalInput")
    var_t = nc.dram_tensor("bn_var", [C], F32, kind="ExternalInput")
    scl_t = nc.dram_tensor("scale", [1], F32, kind="ExternalInput")
    out_t = nc.dram_tensor("out_sim", [PAIRS], F32, kind="ExternalOutput")

    from contextlib import ExitStack

    with tile.TileContext(nc) as tc, ExitStack() as ctx:
        _emit(ctx, tc, nc, spt_t.ap(), qry_t.ap(), w_t.ap(), gam_t.ap(),
              bet_t.ap(), mu_t.ap(), var_t.ap(), scl_t.ap(), out_t.ap())
    nc.compile()
    _dedup_act_table_loads(nc)
    return nc


def _dedup_act_table_loads(nc):
    """Keep one act-table load targeting natural_log_exp_and_others (serves
    Exp, Ln, Relu -- everything this kernel activates)."""
    from concourse.hw_specs import get_activation_tables

    names = list(get_activation_tables(nc.m.arch).keys())
    combined = names.index("natural_log_exp_and_others")
    kept = False
    for b in nc.m.functions[0].blocks:
        keep = []
        for i in b.instructions:
            if type(i).__name__ == "InstLoadActFuncSet":
                si = i.sync_info
                assert si is None or (not si.on_wait and not si.on_update)
                if kept:
                    continue
                i.act_func_set_id = combined
                kept = True
            keep.append(i)
        if len(keep) != len(b.instructions):
            b.instructions[:] = keep


def _ap3(t_ap, dims):
    """Build an AP over tile t_ap with explicit free dims [(stride, n), ...]."""
    return bass.AP(tensor=t_ap.tensor, offset=t_ap.offset,
                   ap=[list(t_ap.ap[0])] + [list(d) for d in dims])


def _bcast_col(t_ap, n):
    """[P,1] AP -> [P,n] stride-0 free-dim broadcast."""
    return bass.AP(tensor=t_ap.tensor, offset=t_ap.offset,
                   ap=[list(t_ap.ap[0]), [0, n]])


def _emit(ctx, tc, nc, spt, qry, conv_w, gam, bet, mu, var, scl, out_sim):
    consts = ctx.enter_context(tc.tile_pool(name="consts", bufs=1))
    pre = ctx.enter_context(tc.tile_pool(name="pre", bufs=2))
    perq = ctx.enter_context(tc.tile_pool(name="perq", bufs=2))
    e0pool = ctx.enter_context(tc.tile_pool(name="e0", bufs=2 * NCH0))
    e1pool = ctx.enter_context(tc.tile_pool(name="e1", bufs=2 * WAY * NCH1))
    # PSUM: 8 banks total
    ps_corr = ctx.enter_context(tc.tile_pool(name="ps_corr", bufs=3, space="PSUM"))
    ps_attq = ctx.enter_context(tc.tile_pool(name="ps_attq", bufs=1, space="PSUM"))
    ps_atts = ctx.enter_context(tc.tile_pool(name="ps_atts", bufs=2, space="PSUM"))
    ps_misc = ctx.enter_context(tc.tile_pool(name="ps_misc", bufs=2, space="PSUM"))

    # ---- constants ----
    ident = consts.tile([128, 128], F32)
    masks.make_identity(nc, ident[:])
    ident_h = consts.tile([128, 128], BF16)
    nc.vector.tensor_copy(ident_h[:], ident[:])
    ones128 = consts.tile([128, 1], F32)
    nc.gpsimd.memset(ones128[:], 1.0)
    ones_h = consts.tile([1, 128], BF16)
    nc.gpsimd.memset(ones_h[:], 1.0)
    oinv_rep = consts.tile([C, C], F32)          # all 1/64 -> channel-mean matmul
    nc.gpsimd.memset(oinv_rep[:], 1.0 / C)
    zeros = consts.tile([128, 1], F32)
    nc.gpsimd.memset(zeros[:], 0.0)
    c25e5 = consts.tile([128, 1], F32)           # bias for stats sqrt: 25*1e-5
    nc.gpsimd.memset(c25e5[:], 25.0e-5)
    c1e5 = consts.tile([128, 1], F32)            # bias for BN sqrt: 1e-5
    nc.gpsimd.memset(c1e5[:], 1.0e-5)

    # way-membership masks for the cq-branch attention: mask_all[p, c*WAY+w]=1
    # iff global support position c*128+p belongs to way w.
    mask_all = consts.tile([128, NCH0 * WAY], BF16)
    nc.gpsimd.memset(mask_all[:], 0.0)
    ones_col_h = consts.tile([128, 1], BF16)
    nc.gpsimd.memset(ones_col_h[:], 1.0)
    for cix in range(NCH0):
        lo, rows = _chunk0(cix)
        r = 0
        while r < rows:
            w = (lo + r) // S
            seg = min(rows - r, (w + 1) * S - (lo + r))
            # mid-partition writes need DMA (engines can't start at p>0 here)
            nc.gpsimd.dma_start(
                out=mask_all[r:r + seg, cix * WAY + w:cix * WAY + w + 1],
                in_=ones_col_h[0:seg])
            r += seg

    # ---- input loads (split per image so consumers start early) ----
    spt_raw = consts.tile([C, SP], F32)
    for w in range(WAY):
        nc.sync.dma_start(out=spt_raw[:, w * S:(w + 1) * S],
                          in_=spt[w])
    qry_raw = consts.tile([C, QPC * S], F32)
    for q in range(QPC):
        nc.sync.dma_start(out=qry_raw[:, q * S:(q + 1) * S],
                          in_=qry[q])
    w_sb = consts.tile([C, C], F32)
    nc.sync.dma_start(out=w_sb[:], in_=conv_w)
    gam_sb = consts.tile([C, 1], F32)
    nc.sync.dma_start(out=gam_sb[:], in_=gam.unsqueeze(1))
    bet_sb = consts.tile([C, 1], F32)
    nc.sync.dma_start(out=bet_sb[:], in_=bet.unsqueeze(1))
    mu_sb = consts.tile([C, 1], F32)
    nc.sync.dma_start(out=mu_sb[:], in_=mu.unsqueeze(1))
    var_sb = consts.tile([C, 1], F32)
    nc.sync.dma_start(out=var_sb[:], in_=var.unsqueeze(1))
    scale_b = consts.tile([25, 1], F32)
    nc.gpsimd.dma_start(
        out=scale_b[:],
        in_=bass.AP(tensor=scl.tensor, offset=scl.offset, ap=[[0, 25], [1, 1]]))

    # conv weight transposed: lhsT layout [c_in, c_out]
    wT_ps = ps_misc.tile([C, C], F32, tag="m")
    nc.tensor.transpose(wT_ps[:], w_sb[:], ident[0:C, 0:C])
    wT_sb = consts.tile([C, C], F32)
    nc.vector.tensor_copy(wT_sb[:], wT_ps[:])

    # BN fold: bns = gamma / sqrt(var + 1e-5);  bnb = beta - mean * bns
    sd = pre.tile([C, 1], F32, tag="bn")
    nc.scalar.activation(sd[:], var_sb[:], AF.Ln, bias=c1e5[0:C], scale=1.0)
    rsd = pre.tile([C, 1], F32, tag="bn")
    nc.scalar.activation(rsd[:], sd[:], AF.Exp, bias=zeros[0:C], scale=-0.5)
    bns = consts.tile([C, 1], F32)
    nc.vector.tensor_mul(bns[:], gam_sb[:], rsd[:])
    mb = pre.tile([C, 1], F32, tag="bn")
    nc.vector.tensor_mul(mb[:], mu_sb[:], bns[:])
    bnb = consts.tile([C, 1], F32)
    nc.vector.scalar_tensor_tensor(out=bnb[:], in0=mb[:], scalar=-1.0,
                                   in1=bet_sb[:], op0=OP.mult, op1=OP.add)

    # fold the channel-mean centering into the conv weight (rank-1 update):
    # y = W(x - xbar) = W'x with W' = W - wbar 1^T, wbar = W @ ones/64.
    # Keeps the hot loop free of fp32 matmuls (fp32 PE ops hold the clock
    # gate at 1.2 GHz).
    wbar_ps = ps_misc.tile([1, C], F32, tag="m")
    nc.tensor.matmul(out=wbar_ps[:], lhsT=oinv_rep[:, 0:1], rhs=wT_sb[:],
                     start=True, stop=True)
    wbarT = consts.tile([1, C], BF16)
    nc.vector.tensor_copy(wbarT[:], wbar_ps[:])
    wrep_ps = ps_misc.tile([C, C], F32, tag="m")
    nc.tensor.matmul(out=wrep_ps[:], lhsT=ones_h[0:1, 0:C], rhs=wbarT[:],
                     start=True, stop=True)
    wc_h = consts.tile([C, C], BF16)
    nc.vector.tensor_sub(wc_h[:], wT_sb[:], wrep_ps[:])

    # ---- channel-mean centering of the support (for cs-branch pooling) ----
    spt_n = consts.tile([C, SP], F32)
    for w in range(WAY):
        mean_ps = ps_misc.tile([C, S], F32, tag="m")
        nc.tensor.matmul(out=mean_ps[:], lhsT=oinv_rep[:],
                         rhs=spt_raw[:, w * S:(w + 1) * S], start=True, stop=True)
        nc.vector.tensor_sub(spt_n[:, w * S:(w + 1) * S],
                             spt_raw[:, w * S:(w + 1) * S], mean_ps[:])
    # raw support in bf16 (feature-transform input; centering is in W')
    sptb = consts.tile([C, SP], BF16)
    nc.vector.tensor_copy(sptb[:], spt_raw[:])

    # ---- feature transform: conv+bn+relu, L2-normalize over channels, Gram ----
    def feat_transform(x_slice, h_out, gc_out, hT_out):
        """x_slice: [C, S] bf16 RAW features (centering folded into wc_h);
        h_out: [C, S] bf16 AP for the normalized features (channel-major);
        gc_out: [C, C] bf16 AP for the centered Gram; hT_out: [CH1, NCH1*C]
        bf16 tile (position-major, 100-chunks) or None."""
        y_ps = ps_misc.tile([C, S], F32, tag="m")
        nc.tensor.matmul(out=y_ps[:], lhsT=wc_h[:], rhs=x_slice,
                         start=True, stop=True)
        bnr = pre.tile([C, S], BF16, tag="bnr")
        nc.scalar.activation(bnr[:], y_ps[:], AF.Relu, bias=bnb[:], scale=bns[:])
        hT_raw = pre.tile([CH1, NCH1 * C], BF16, tag="hTraw")
        nsq = pre.tile([CH1, NCH1], F32, tag="nsq")
        for j in range(NCH1):
            tp_ps = ps_misc.tile([CH1, C], BF16, tag="m")
            nc.tensor.transpose(tp_ps[:], bnr[:, j * CH1:(j + 1) * CH1],
                                ident_h[0:C, 0:C])
            nc.vector.tensor_copy(hT_raw[:, j * C:(j + 1) * C], tp_ps[:])
            sqscr = pre.tile([CH1, C], F32, tag="sqscr")
            nc.vector.scalar_tensor_tensor(
                out=sqscr[:], in0=hT_raw[:, j * C:(j + 1) * C], scalar=1.0,
                in1=hT_raw[:, j * C:(j + 1) * C],
                op0=OP.mult, op1=OP.mult, accum_out=nsq[:, j:j + 1])
        nc.vector.tensor_scalar_max(nsq[:], nsq[:], 1.0e-16)
        nrm = pre.tile([CH1, NCH1], F32, tag="nrm")
        nc.scalar.activation(nrm[:], nsq[:], AF.Ln, bias=zeros[0:CH1], scale=1.0)
        rinv = pre.tile([CH1, NCH1], F32, tag="rinv")
        nc.scalar.activation(rinv[:], nrm[:], AF.Exp, bias=zeros[0:CH1], scale=-0.5)
        hT_sc = hT_out if hT_out is not None else pre.tile(
            [CH1, NCH1 * C], BF16, tag="hTsc")
        for j in range(NCH1):
            nc.vector.tensor_scalar_mul(hT_sc[:, j * C:(j + 1) * C],
                                        hT_raw[:, j * C:(j + 1) * C], rinv[:, j:j + 1])
        # normalized features back to channel-major + spatial row-sum accum
        rowp = pre.tile([C, NCH1], F32, tag="rowp")
        for j in range(NCH1):
            bk_ps = ps_misc.tile([C, CH1], BF16, tag="m")
            nc.tensor.transpose(bk_ps[:], hT_sc[:, j * C:(j + 1) * C],
                                ident_h[0:CH1, 0:CH1])
            nc.vector.tensor_scalar(
                out=h_out[:, j * CH1:(j + 1) * CH1], in0=bk_ps[:],
                scalar1=1.0, scalar2=None, op0=OP.mult, op1=OP.add,
                accum_out=rowp[:, j:j + 1])
        # centered Gram: Gc = sum_p (h_p - hbar)(h_p - hbar)^T
        hbar = pre.tile([C, 1], F32, tag="hbar")
        nc.vector.tensor_reduce(out=hbar[:], in_=rowp[:], axis=AX.X, op=OP.add)
        hbar_b = pre.tile([C, 1], BF16, tag="hbarb")
        nc.vector.tensor_copy(hbar_b[:], hbar[:])
        hbT_ps = ps_misc.tile([1, C], BF16, tag="m")
        nc.tensor.transpose(hbT_ps[:], hbar_b[:], ident_h[0:C, 0:C])
        hbar_h = pre.tile([1, C], BF16, tag="hbarh")
        nc.vector.tensor_scalar_mul(hbar_h[:], hbT_ps[:], 1.0 / S)
        hb_ps = ps_misc.tile([CH1, C], F32, tag="m")
        nc.tensor.matmul(out=hb_ps[:], lhsT=ones_h[0:1, 0:CH1], rhs=hbar_h[:],
                         start=True, stop=True)
        hTc = pre.tile([CH1, NCH1 * C], BF16, tag="hTc")
        for j in range(NCH1):
            nc.vector.tensor_sub(hTc[:, j * C:(j + 1) * C],
                                 hT_sc[:, j * C:(j + 1) * C], hb_ps[:])
        Gc_ps = ps_misc.tile([C, C], F32, tag="m")
        for j in range(NCH1):
            nc.tensor.matmul(out=Gc_ps[:], lhsT=hTc[:, j * C:(j + 1) * C],
                             rhs=hTc[:, j * C:(j + 1) * C],
                             start=(j == 0), stop=(j == NCH1 - 1))
        nc.vector.tensor_copy(gc_out, Gc_ps[:])

    # ---- support features (once, stage-batched across all 5 ways so the
    # engines overlap instead of walking one way's serial chain at a time) ----
    s_all = consts.tile([C, SP], BF16)            # normalized feats, all ways
    gc_s5 = consts.tile([C, WAY * C], BF16)       # 5 centered Grams
    NB = SP // 500                                # 4 psum-bank batches
    NCK = SP // CH1                               # 20 position chunks
    bnr_all = consts.tile([C, SP], BF16)
    for b in range(NB):
        y_ps = ps_misc.tile([C, 500], F32, tag="m")
        nc.tensor.matmul(out=y_ps[:], lhsT=wc_h[:],
                         rhs=sptb[:, b * 500:(b + 1) * 500], start=True, stop=True)
        nc.scalar.activation(bnr_all[:, b * 500:(b + 1) * 500], y_ps[:],
                             AF.Relu, bias=bnb[:], scale=bns[:])
    shT_raw = consts.tile([CH1, NCK * C], BF16)
    nsq_all = consts.tile([CH1, NCK], F32)
    for j in range(NCK):
        tp_ps = ps_misc.tile([CH1, C], BF16, tag="m")
        nc.tensor.transpose(tp_ps[:], bnr_all[:, j * CH1:(j + 1) * CH1],
                            ident_h[0:C, 0:C])
        nc.vector.tensor_copy(shT_raw[:, j * C:(j + 1) * C], tp_ps[:])
    for j in range(NCK):
        sqscr = pre.tile([CH1, C], F32, tag="sqscr")
        nc.vector.scalar_tensor_tensor(
            out=sqscr[:], in0=shT_raw[:, j * C:(j + 1) * C], scalar=1.0,
            in1=shT_raw[:, j * C:(j + 1) * C],
            op0=OP.mult, op1=OP.mult, accum_out=nsq_all[:, j:j + 1])
    nc.vector.tensor_scalar_max(nsq_all[:], nsq_all[:], 1.0e-16)
    snrm = pre.tile([CH1, NCK], F32, tag="snrm")
    nc.scalar.activation(snrm[:], nsq_all[:], AF.Ln, bias=zeros[0:CH1], scale=1.0)
    srinv = pre.tile([CH1, NCK], F32, tag="srinv")
    nc.scalar.activation(srinv[:], snrm[:], AF.Exp, bias=zeros[0:CH1], scale=-0.5)
    shT_sc = consts.tile([CH1, NCK * C], BF16)
    for j in range(NCK):
        nc.vector.tensor_scalar_mul(shT_sc[:, j * C:(j + 1) * C],
                                    shT_raw[:, j * C:(j + 1) * C],
                                    srinv[:, j:j + 1])
    srowp = consts.tile([C, NCK], F32)
    for j in range(NCK):
        bk_ps = ps_misc.tile([C, CH1], BF16, tag="m")
        nc.tensor.transpose(bk_ps[:], shT_sc[:, j * C:(j + 1) * C],
                            ident_h[0:CH1, 0:CH1])
        nc.vector.tensor_scalar(
            out=s_all[:, j * CH1:(j + 1) * CH1], in0=bk_ps[:],
            scalar1=1.0, scalar2=None, op0=OP.mult, op1=OP.add,
            accum_out=srowp[:, j:j + 1])
    # per-way spatial means -> centered chunks -> Grams
    hbar5 = pre.tile([C, WAY], F32, tag="hbar5")
    nc.vector.tensor_reduce(out=hbar5[:],
                            in_=_ap3(srowp[:], [[NCH1, WAY], [1, NCH1]]),
                            axis=AX.X, op=OP.add)
    hbar5b = pre.tile([C, WAY], BF16, tag="hbar5b")
    nc.vector.tensor_copy(hbar5b[:], hbar5[:])
    hbT5 = pre.tile([1, WAY * C], BF16, tag="hbT5")
    for w in range(WAY):
        hbT_ps = ps_misc.tile([1, C], BF16, tag="m")
        nc.tensor.transpose(hbT_ps[:], hbar5b[:, w:w + 1], ident_h[0:C, 0:C])
        nc.vector.tensor_scalar_mul(hbT5[:, w * C:(w + 1) * C], hbT_ps[:], 1.0 / S)
    shTc = consts.tile([CH1, NCK * C], BF16)
    for w in range(WAY):
        hb_ps = ps_misc.tile([CH1, C], F32, tag="m")
        nc.tensor.matmul(out=hb_ps[:], lhsT=ones_h[0:1, 0:CH1],
                         rhs=hbT5[:, w * C:(w + 1) * C], start=True, stop=True)
        for k in range(NCH1):
            j = w * NCH1 + k
            nc.vector.tensor_sub(shTc[:, j * C:(j + 1) * C],
                                 shT_sc[:, j * C:(j + 1) * C], hb_ps[:])
    for w in range(WAY):
        Gc_ps = ps_misc.tile([C, C], F32, tag="m")
        for k in range(NCH1):
            j = w * NCH1 + k
            nc.tensor.matmul(out=Gc_ps[:], lhsT=shTc[:, j * C:(j + 1) * C],
                             rhs=shTc[:, j * C:(j + 1) * C],
                             start=(k == 0), stop=(k == NCH1 - 1))
        nc.vector.tensor_copy(gc_s5[:, w * C:(w + 1) * C], Gc_ps[:])

    # position-major support features, 128-chunks across ways (for cq stt)
    hT_all = consts.tile([128, NCH0 * C], BF16)
    for cix in range(NCH0):
        lo, rows = _chunk0(cix)
        tp = ps_misc.tile([128, C], BF16, tag="m")
        nc.tensor.transpose(tp[0:rows], s_all[:, lo:lo + rows], ident_h[0:C, 0:C])
        nc.vector.tensor_copy(hT_all[0:rows, cix * C:(cix + 1) * C], tp[0:rows])

    # attention/pooling accumulators over all pairs
    sall = consts.tile([C, PAIRS], F32)   # spt_att columns (pair-major q*5+w)
    qall = consts.tile([C, PAIRS], F32)   # qry_att columns
    sims = consts.tile([25, 2], F32)   # col h = pairs h*25..h*25+24
    attqT = consts.tile([128, 4 * WAY], BF16)     # attn_q^T chunks (per query)
    nc.gpsimd.memset(attqT[:], 0.0)

    # per-query persistent feature arrays (all 10 queries computed upfront,
    # interleaved with the 5 support transforms so every engine stays busy
    # during the startup phase and the hot loop stays pure corr->exp->attn)
    qh_all = consts.tile([C, QPC * S], BF16)
    gcq_all = consts.tile([C, QPC * C], BF16)
    qhT_all = consts.tile([CH1, QPC * NCH1 * C], BF16)
    qnT_all = consts.tile([128, QPC * 4 * C], BF16)

    # ---------------- per-query pipeline ----------------
    qctx = {}

    def emit_feat(q):
        d = {}
        # raw bf16 features, 512-padded for the DMA transpose
        qrb = pre.tile([C, 512], BF16, tag="qrb", name="qrb")
        nc.gpsimd.memset(qrb[:, S:512], 0.0)
        nc.vector.tensor_copy(qrb[:, 0:S], qry_raw[:, q * S:(q + 1) * S])
        d["qh"] = qh_all[:, q * S:(q + 1) * S]
        d["gcq"] = gcq_all[:, q * C:(q + 1) * C]
        d["qhT"] = qhT_all[:, q * NCH1 * C:(q + 1) * NCH1 * C]
        feat_transform(qrb[:, 0:S], d["qh"], d["gcq"], d["qhT"])
        # position-major raw features via DMA transpose, then channel-mean
        # centering per position (row) on DVE: qnT = qnT_raw - rowmean.
        qnTr = pre.tile([128, 4 * C], BF16, tag="qnTr", name="qnTr")
        for j in range(4):
            nc.sync.dma_start_transpose(qnTr[:, j * C:(j + 1) * C],
                                        qrb[:, j * 128:(j + 1) * 128])
        d["qnT"] = qnT_all[:, q * 4 * C:(q + 1) * 4 * C]
        qmean = pre.tile([128, 4], F32, tag="qmean", name="qmean")
        for j in range(4):
            nc.vector.tensor_reduce(out=qmean[:, j:j + 1],
                                    in_=qnTr[:, j * C:(j + 1) * C],
                                    axis=AX.X, op=OP.add)
        nc.vector.tensor_scalar_mul(qmean[:], qmean[:], 1.0 / C)
        for j in range(4):
            nc.vector.tensor_scalar(
                out=d["qnT"][:, j * C:(j + 1) * C],
                in0=qnTr[:, j * C:(j + 1) * C],
                scalar1=qmean[:, j:j + 1], scalar2=None,
                op0=OP.subtract, op1=OP.bypass)
        qctx[q] = d

    def emit_uvar(q):
        """All 36 variance quadratic forms + batched rr for query q.
        stt layout: [128, NCH0 + WAY*NCH1]; cols 0:16 = cq chunks (rows =
        chunk rows), cols 16:36 = cs (way,chunk) (rows 0:100)."""
        d = qctx[q]
        stt = perq.tile([128, NUNITS], F32, tag="stt", name="stt")
        nc.gpsimd.memset(stt[:], 0.0)
        for cix in range(NCH0):
            lo, rows = _chunk0(cix)
            u_ps = ps_misc.tile([128, C], F32, tag="m")
            nc.tensor.matmul(out=u_ps[0:rows], lhsT=s_all[:, lo:lo + rows],
                             rhs=d["gcq"], start=True, stop=True)
            scr = pre.tile([128, C], F32, tag="uscr")
            nc.vector.scalar_tensor_tensor(
                out=scr[0:rows], in0=u_ps[0:rows], scalar=1.0,
                in1=hT_all[0:rows, cix * C:(cix + 1) * C],
                op0=OP.mult, op1=OP.mult, accum_out=stt[0:rows, cix:cix + 1])
        qhTq = d["qhT"]
        for j in range(NCH1):
            u5_ps = ps_misc.tile([CH1, WAY * C], F32, tag="m")
            nc.tensor.matmul(out=u5_ps[:],
                             lhsT=d["qh"][:, j * CH1:(j + 1) * CH1],
                             rhs=gc_s5[:], start=True, stop=True)
            for w in range(WAY):
                scr = pre.tile([CH1, C], F32, tag="uscr")
                col = NCH0 + j * WAY + w
                nc.vector.scalar_tensor_tensor(
                    out=scr[:], in0=u5_ps[:, w * C:(w + 1) * C], scalar=1.0,
                    in1=qhTq[:, j * C:(j + 1) * C],
                    op0=OP.mult, op1=OP.mult, accum_out=stt[0:CH1, col:col + 1])
        # rr = 1/(TEMP*sqrt(var+1e-5)), var = stt/399; junk rows stay finite.
        sq = perq.tile([128, NUNITS], F32, tag="sq", name="sq")
        nc.scalar.activation(sq[:], stt[:], AF.Ln, bias=c25e5[:],
                             scale=(TEMP * TEMP) / (S - 1.0))
        rr = perq.tile([128, NUNITS], F32, tag="rr", name="rr")
        nc.scalar.activation(rr[:], sq[:], AF.Exp, bias=zeros[:], scale=-0.5)
        d["rr"] = rr
        d["z"] = perq.tile([128, NUNITS], F32, tag="z", name="z")
        d["E0"] = [None] * NCH0
        d["E1"] = [None] * (WAY * NCH1)

    # unit list: interleave cq chunks and cs (way, chunk) units
    units = []
    u0 = [("cq", cix) for cix in range(NCH0)]
    u1 = [("cs", w * NCH1 + k) for w in range(WAY) for k in range(NCH1)]
    i0 = i1 = 0
    for i in range(NUNITS):
        # ratio 16:20 -> alternate with slight cs surplus
        if (i * NCH0) // NUNITS >= i0 + (1 if i1 > i0 else 0) and i0 < NCH0:
            units.append(u0[i0]); i0 += 1
        elif i1 < len(u1):
            units.append(u1[i1]); i1 += 1
        else:
            units.append(u0[i0]); i0 += 1

    def emit_unit(q, ui):
        """One corr matmul + exp (+ denominator) unit."""
        d = qctx[q]
        kind, ix = units[ui]
        on_dve = ui < N_DENOM_DVE  # spread: first units' denoms on DVE
        if kind == "cq":
            cix = ix
            lo, rows = _chunk0(cix)
            cp = ps_corr.tile([128, S], F32, tag="corr")
            nc.tensor.matmul(out=cp[0:rows], lhsT=s_all[:, lo:lo + rows],
                             rhs=d["qh"][:], start=True, stop=True)
            e = e0pool.tile([128, S], BF16, tag="E0", name="E0")
            col = cix
            zcol = d["z"][0:rows, col:col + 1]
            if on_dve:
                nc.scalar.activation(e[0:rows], cp[0:rows], AF.Exp,
                                     bias=zeros[0:rows],
                                     scale=d["rr"][0:rows, col:col + 1])
                nc.vector.tensor_reduce(out=zcol, in_=e[0:rows], axis=AX.X,
                                        op=OP.add)
            else:
                nc.scalar.activation(e[0:rows], cp[0:rows], AF.Exp,
                                     bias=zeros[0:rows],
                                     scale=d["rr"][0:rows, col:col + 1],
                                     accum_out=zcol)
            d["E0"][cix] = e
        else:
            w, k = ix // NCH1, ix % NCH1
            cp = ps_corr.tile([128, S], F32, tag="corr")
            nc.tensor.matmul(out=cp[0:CH1],
                             lhsT=d["qh"][:, k * CH1:(k + 1) * CH1],
                             rhs=s_all[:, w * S:(w + 1) * S], start=True, stop=True)
            e = e1pool.tile([CH1, S], BF16, tag="E1", name="E1")
            col = NCH0 + k * WAY + w
            zcol = d["z"][0:CH1, col:col + 1]
            if on_dve:
                nc.scalar.activation(e[:], cp[0:CH1], AF.Exp, bias=zeros[0:CH1],
                                     scale=d["rr"][0:CH1, col:col + 1])
                nc.vector.tensor_reduce(out=zcol, in_=e[:], axis=AX.X, op=OP.add)
            else:
                nc.scalar.activation(e[:], cp[0:CH1], AF.Exp, bias=zeros[0:CH1],
                                     scale=d["rr"][0:CH1, col:col + 1],
                                     accum_out=zcol)
            d["E1"][ix] = e

    def tail_ops(q):
        """Attention + pooling for query q as a list of thunks (emitted
        interleaved into the next query's exp stream)."""
        d = qctx[q]
        ops = []

        def recips():
            wrec = perq.tile([128, NUNITS], BF16, tag="wrec", name="wrec")
            with nc.allow_low_precision(reason="bf16 softmax weights"):
                nc.vector.reciprocal(wrec[:], d["z"][:])
            d["wrec"] = wrec
            # masked cq attention weights in ONE stride-0 bcast op:
            # mw[p, c, w] = mask_all[p, c, w] * wrec[p, c]
            mw = perq.tile([128, NCH0 * WAY], BF16, tag="mw", name="mw")
            nc.vector.tensor_mul(
                _ap3(mw[:], [[WAY, NCH0], [1, WAY]]),
                _ap3(mask_all[:], [[WAY, NCH0], [1, WAY]]),
                _ap3(wrec[:], [[1, NCH0], [0, WAY]]))
            d["mw"] = mw
        ops.append(recips)

        # cq attention: 16 masked matmuls accumulate attn_q [5, 400]
        attq = {}

        def mk_cq_attn(cix, first, last):
            def f():
                if first:
                    attq["t"] = ps_attq.tile([WAY, S], F32, tag="attq", name="attq")
                lo, rows = _chunk0(cix)
                nc.tensor.matmul(
                    out=attq["t"][:],
                    lhsT=d["mw"][0:rows, cix * WAY:(cix + 1) * WAY],
                    rhs=d["E0"][cix][0:rows], start=first, stop=last)
            return f
        for cix in range(NCH0):
            ops.append(mk_cq_attn(cix, cix == 0, cix == NCH0 - 1))

        def attq_tail():
            # attn_q -> SBUF bf16, transpose 128-pieces, pool via PE
            aq = perq.tile([WAY, S], BF16, tag="aq", name="aq")
            nc.vector.tensor_copy(aq[:], attq["t"][:])
            d["aq"] = aq
        ops.append(attq_tail)

        def mk_attq_tp(j):
            def f():
                npc = min(128, S - j * 128)
                tp = ps_misc.tile([128, WAY], BF16, tag="m")
                nc.tensor.transpose(tp[0:npc], d["aq"][:, j * 128:j * 128 + npc],
                                    ident_h[0:WAY, 0:WAY])
                nc.vector.tensor_copy(attqT[0:npc, j * WAY:(j + 1) * WAY],
                                      tp[0:npc])
            return f
        for j in range(4):
            ops.append(mk_attq_tp(j))

        qatt = {}

        def mk_qpool(j, first, last):
            def f():
                if first:
                    qatt["t"] = ps_misc.tile([C, WAY], F32, tag="m", name="qatt")
                npc = min(128, S - j * 128)
                nc.tensor.matmul(out=qatt["t"][:],
                                 lhsT=qctx[q]["qnT"][0:npc, j * C:(j + 1) * C],
                                 rhs=attqT[0:npc, j * WAY:(j + 1) * WAY],
                                 start=first, stop=last)
                if last:
                    nc.vector.tensor_copy(qall[:, q * WAY:(q + 1) * WAY],
                                          qatt["t"][:])
            return f
        for j in range(4):
            ops.append(mk_qpool(j, j == 0, j == 3))

        # cs attention per way: bcast-column 1/Z lhsT, pool on DVE
        atts = {}

        def mk_cs_attn(w, k, first, last):
            def f():
                if first:
                    atts["t"] = ps_atts.tile([C, S], F32, tag="atts", name="atts")
                col = NCH0 + k * WAY + w
                nc.tensor.matmul(
                    out=atts["t"][:],
                    lhsT=_bcast_col(d["wrec"][0:CH1, col:col + 1], C),
                    rhs=d["E1"][w * NCH1 + k][:], start=first, stop=last)
                if last:
                    pj = pre.tile([C, S], F32, tag="pj")
                    nc.vector.scalar_tensor_tensor(
                        out=pj[:], in0=spt_n[:, w * S:(w + 1) * S],
                        scalar=1.0, in1=atts["t"][:], op0=OP.mult, op1=OP.mult,
                        accum_out=sall[:, q * WAY + w:q * WAY + w + 1])
            return f
        for w in range(WAY):
            for k in range(NCH1):
                ops.append(mk_cs_attn(w, k, k == 0, k == NCH1 - 1))

        def cleanup():
            qctx.pop(q, None)
        ops.append(cleanup)
        return ops

    def emit_cosine(plo, phi):
        n = phi - plo
        h = plo // 25
        p3 = pre.tile([C, 3 * PAIRS], F32, tag="p3", name="p3")
        nc.vector.tensor_mul(p3[:, 0:n], sall[:, plo:phi], qall[:, plo:phi])
        nc.vector.tensor_mul(p3[:, n:2 * n], sall[:, plo:phi], sall[:, plo:phi])
        nc.vector.tensor_mul(p3[:, 2 * n:3 * n], qall[:, plo:phi], qall[:, plo:phi])
        dots = ps_misc.tile([PAIRS, 3], F32, tag="m", name="dots")
        for i in range(3):
            nc.tensor.matmul(out=dots[0:n, i:i + 1],
                             lhsT=p3[:, i * n:(i + 1) * n],
                             rhs=ones128[0:C], start=True, stop=True)
        nrm2 = pre.tile([PAIRS, 2], F32, tag="nrm2", name="nrm2")
        nc.vector.tensor_scalar_max(nrm2[0:n], dots[0:n, 1:3], 1.6e-7)
        lnn = pre.tile([PAIRS, 2], F32, tag="lnn", name="lnn")
        nc.scalar.activation(lnn[0:n], nrm2[0:n], AF.Ln, bias=zeros[0:n], scale=1.0)
        lsum = pre.tile([PAIRS, 1], F32, tag="lsum", name="lsum")
        nc.vector.tensor_add(lsum[0:n], lnn[0:n, 0:1], lnn[0:n, 1:2])
        rden = pre.tile([PAIRS, 1], F32, tag="rden", name="rden")
        nc.scalar.activation(rden[0:n], lsum[0:n], AF.Exp, bias=zeros[0:n], scale=-0.5)
        s0 = pre.tile([PAIRS, 1], F32, tag="s0", name="s0")
        nc.vector.tensor_mul(s0[0:n], dots[0:n, 0:1], rden[0:n])
        nc.vector.tensor_mul(sims[0:n, h:h + 1], s0[0:n], scale_b[0:n])

    # ---------------- top-level schedule ----------------
    emit_feat(0)
    emit_feat(1)
    emit_uvar(0)
    pending = []           # tail thunks of the previous query
    for q in range(QPC):
        for ui in range(NUNITS):
            emit_unit(q, ui)
            # spread previous query's attention/pool tail into this stream
            take = (len(pending) + NUNITS - 1 - ui) // (NUNITS - ui)
            for _ in range(take):
                pending.pop(0)()
            # next-query prep at fixed points
            if q + 2 < QPC and ui == 8:
                emit_feat(q + 2)
            if q + 1 < QPC and ui == 22:
                emit_uvar(q + 1)
        pending = tail_ops(q)
        if q == 5:
            pending.append(lambda: emit_cosine(0, 25))
    for f in pending:
        f()
    emit_cosine(25, PAIRS)
    nc.sync.dma_start(
        out=bass.AP(tensor=out_sim.tensor, offset=out_sim.offset,
                    ap=[[1, 25], [25, 2]]),
        in_=sims[:])


_PROGRAM = None


def _get_program():
    global _PROGRAM
    if _PROGRAM is None:
        _PROGRAM = _build_program()
    return _PROGRAM


def kernel(spt, qry, conv_w, bn_gamma, bn_beta, bn_mean, bn_var, scale):
    spt = np.ascontiguousarray(np.asarray(spt, dtype=np.float32).reshape(WAY, C, S))
    qry = np.asarray(qry, dtype=np.float32).reshape(-1, C, S)
    nq = qry.shape[0]
    npad = NCORES * QPC
    qpad = np.zeros((npad, C, S), dtype=np.float32)
    qpad[:nq] = qry
    in_maps = []
    for core in range(NCORES):
        in_maps.append({
            "spt": spt,
            "qry": np.ascontiguousarray(qpad[core * QPC:(core + 1) * QPC]),
            "conv_w": np.asarray(conv_w, dtype=np.float32),
            "bn_gamma": np.asarray(bn_gamma, dtype=np.float32),
            "bn_beta": np.asarray(bn_beta, dtype=np.float32),
            "bn_mean": np.asarray(bn_mean, dtype=np.float32),
            "bn_var": np.asarray(bn_var, dtype=np.float32),
            "scale": np.asarray(scale, dtype=np.float32),
        })
    nc = _get_program()
    trace = bool(os.environ.get("KBENCH_TRACE"))
    kw = {}
    if trace:
        import tempfile
        kw = dict(trace=True, tmpdir=tempfile.mkdtemp(prefix="ktrace_"))
    res = run_bass_kernel_spmd(nc, in_maps, list(range(NCORES)), **kw)
    if trace:
        global LAST_RESULTS
        LAST_RESULTS = res
        print("exec_time_ns:", res.exec_time_ns,
              "mean:", res.mean_exec_time_ns,
              "worst core:", res.max_exec_time_core_id)
        if res.instructions_and_trace:
            print("trace path:", res.instructions_and_trace[1])
    outs = [np.asarray(res.results[i]["out_sim"]) for i in range(NCORES)]
    full = np.concatenate([o.reshape(QPC, WAY) for o in outs], axis=0)  # [80, 5]
    return np.ascontiguousarray(full[:nq]).astype(np.float32)


if __name__ == "__main__":
    rng = np.random.default_rng(0)
    ins = {
        "spt": rng.standard_normal((WAY, C, 20, 20), dtype=np.float32),
        "qry": rng.standard_normal((75, C, 20, 20), dtype=np.float32),
        "conv_w": (rng.standard_normal((C, C)) * 0.1).astype(np.float32),
        "bn_gamma": np.ones(C, np.float32),
        "bn_beta": np.zeros(C, np.float32),
        "bn_mean": np.zeros(C, np.float32),
        "bn_var": np.ones(C, np.float32),
        "scale": np.ones(1, np.float32),
    }
    out = kernel(**ins)
    print(out.shape, out.dtype, out[:2])


# revision 30
# speedup vs baseline: 1.2074x; 1.0552x over previous
"""Trainium2 Bass kernel for nn_Backbone_47390669144486 (SAM-style 4D-correlation attention).

Data-parallel over the 75 queries across 8 NeuronCores (pad to 80, 10/core).
Restructured from the chunk-of-100-per-(query,way) baseline into a phased
per-query pipeline that keeps ScalarE (the bottleneck: 16M softmax exps/core)
dense and minimizes per-instruction overheads:

  * branch cq (normalize over the query axis): support positions of ALL 5
    ways are flattened to one 2000-length axis and chunked by 128 -> 16
    corr matmuls / exps per query (vs 20), full 128 ACT lanes.
  * the gauss-norm variance is computed UPFRONT from the centered-Gram
    quadratic form (var[p] = h_p^T Gc h_p) via small [*,64]/[*,320]
    matmuls + one DVE rowsum per chunk, so all 36 1/(T*sigma) factors of a
    query batch into TWO Ln/Exp activations (vs 40 small ACTs per query).
  * softmax denominators: fused accum_out on most exps (free row-sum on
    ScalarE, costs one 287ns accumulator drain each); a tunable subset is
    computed by DVE tensor_reduce instead to balance the two engines.
  * attention is deferred to a per-query tail: branch-cq uses a masked
    weight tile [128,5] (way-membership mask x 1/Z, built in ONE stride-0
    broadcast DVE op) so all 16 chunks accumulate attn_q for all 5 ways
    into a single [5,400] PSUM bank; pooling against the centered query
    features runs on the PE using a DMA-transposed [400,64] feature copy.
  * branch cs keeps per-(way, qchunk-100) layout (its softmax axis is the
    support dim of one way); attention uses the stride-0 bcast-column
    1/Z lhsT and pools on DVE as before.
  * cosine similarities for all 50 (query,way) pairs batch into one tail.

All hot-loop matmul operands are bf16; exp outputs are bf16.
"""

import os
import sys

sys.path.insert(0, "/opt/trn_rl_repo")

import numpy as np

import concourse.bass as bass
import concourse.tile as tile
from concourse import bacc, masks, mybir
from concourse.bass_utils import run_bass_kernel_spmd

F32 = mybir.dt.float32
BF16 = mybir.dt.bfloat16
AF = mybir.ActivationFunctionType
OP = mybir.AluOpType
AX = mybir.AxisListType

WAY = 5
C = 64
S = 400            # 20*20 spatial positions
SP = WAY * S       # 2000 flattened support positions
CH0 = 128          # branch-cq chunk (support axis, crosses ways)
NCH0 = (SP + CH0 - 1) // CH0   # 16 (last chunk 80)
CH1 = 100          # branch-cs chunk (query axis)
NCH1 = S // CH1    # 4
NUNITS = NCH0 + WAY * NCH1     # 36 exp units per query
NCORES = 8
QPC = 10
PAIRS = WAY * QPC  # 50
TEMP = 5.0

# how many of the 36 denominators per query go to DVE tensor_reduce
# instead of ScalarE accum_out (engine balancing).
N_DENOM_DVE = 11


def _chunk0(c):
    lo = c * CH0
    return lo, min(SP, lo + CH0) - lo  # (start, rows)


def _build_program():
    nc = bacc.Bacc("TRN2", target_bir_lowering=False, debug=False)

    spt_t = nc.dram_tensor("spt", [WAY, C, S], F32, kind="ExternalInput")
    qry_t = nc.dram_tensor("qry", [QPC, C, S], F32, kind="ExternalInput")
    w_t = nc.dram_tensor("conv_w", [C, C], F32, kind="ExternalInput")
    gam_t = nc.dram_tensor("bn_gamma", [C], F32, kind="ExternalInput")
    bet_t = nc.dram_tensor("bn_beta", [C], F32, kind="ExternalInput")
    mu_t = nc.dram_tensor("bn_mean", [C], F32, kind="ExternalInput")
    var_t = nc.dram_tensor("bn_var", [C], F32, kind="ExternalInput")
    scl_t = nc.dram_tensor("scale", [1], F32, kind="ExternalInput")
    out_t = nc.dram_tensor("out_sim", [PAIRS], F32, kind="ExternalOutput")

    from contextlib import ExitStack

    with tile.TileContext(nc) as tc, ExitStack() as ctx:
        _emit(ctx, tc, nc, spt_t.ap(), qry_t.ap(), w_t.ap(), gam_t.ap(),
              bet_t.ap(), mu_t.ap(), var_t.ap(), scl_t.ap(), out_t.ap())
    nc.compile()
    _dedup_act_table_loads(nc)
    return nc


def _dedup_act_table_loads(nc):
    """Keep one act-table load targeting natural_log_exp_and_others (serves
    Exp, Ln, Relu -- everything this kernel activates)."""
    from concourse.hw_specs import get_activation_tables

    names = list(get_activation_tables(nc.m.arch).keys())
    combined = names.index("natural_log_exp_and_others")
    kept = False
    for b in nc.m.functions[0].blocks:
        keep = []
        for i in b.instructions:
            if type(i).__name__ == "InstLoadActFuncSet":
                si = i.sync_info
                assert si is None or (not si.on_wait and not si.on_update)
                if kept:
                    continue
                i.act_func_set_id = combined
                kept = True
            keep.append(i)
        if len(keep) != len(b.instructions):
            b.instructions[:] = keep


def _ap3(t_ap, dims):
    """Build an AP over tile t_ap with explicit free dims [(stride, n), ...]."""
    return bass.AP(tensor=t_ap.tensor, offset=t_ap.offset,
                   ap=[list(t_ap.ap[0])] + [list(d) for d in dims])


def _bcast_col(t_ap, n):
    """[P,1] AP -> [P,n] stride-0 free-dim broadcast."""
    return bass.AP(tensor=t_ap.tensor, offset=t_ap.offset,
                   ap=[list(t_ap.ap[0]), [0, n]])


def _emit(ctx, tc, nc, spt, qry, conv_w, gam, bet, mu, var, scl, out_sim):
    consts = ctx.enter_context(tc.tile_pool(name="consts", bufs=1))
    pre = ctx.enter_context(tc.tile_pool(name="pre", bufs=2))
    perq = ctx.enter_context(tc.tile_pool(name="perq", bufs=2))
    e0pool = ctx.enter_context(tc.tile_pool(name="e0", bufs=2 * NCH0))
    e1pool = ctx.enter_context(tc.tile_pool(name="e1", bufs=2 * WAY * NCH1))
    # PSUM: 8 banks total
    ps_corr = ctx.enter_context(tc.tile_pool(name="ps_corr", bufs=3, space="PSUM"))
    ps_attq = ctx.enter_context(tc.tile_pool(name="ps_attq", bufs=1, space="PSUM"))
    ps_atts = ctx.enter_context(tc.tile_pool(name="ps_atts", bufs=2, space="PSUM"))
    ps_misc = ctx.enter_context(tc.tile_pool(name="ps_misc", bufs=2, space="PSUM"))

    # ---- constants ----
    ident = consts.tile([128, 128], F32)
    masks.make_identity(nc, ident[:])
    ident_h = consts.tile([128, 128], BF16)
    nc.vector.tensor_copy(ident_h[:], ident[:])
    ones128 = consts.tile([128, 1], F32)
    nc.gpsimd.memset(ones128[:], 1.0)
    ones_h = consts.tile([1, 128], BF16)
    nc.gpsimd.memset(ones_h[:], 1.0)
    oinv_rep = consts.tile([C, C], F32)          # all 1/64 -> channel-mean matmul
    nc.gpsimd.memset(oinv_rep[:], 1.0 / C)
    zeros = consts.tile([128, 1], F32)
    nc.gpsimd.memset(zeros[:], 0.0)
    c25e5 = consts.tile([128, 1], F32)           # bias for stats sqrt: 25*1e-5
    nc.gpsimd.memset(c25e5[:], 25.0e-5)
    c1e5 = consts.tile([128, 1], F32)            # bias for BN sqrt: 1e-5
    nc.gpsimd.memset(c1e5[:], 1.0e-5)

    # way-membership masks for the cq-branch attention: mask_all[p, c*WAY+w]=1
    # iff global support position c*128+p belongs to way w.
    mask_all = consts.tile([128, NCH0 * WAY], BF16)
    nc.gpsimd.memset(mask_all[:], 0.0)
    ones_col_h = consts.tile([128, 1], BF16)
    nc.gpsimd.memset(ones_col_h[:], 1.0)
    for cix in range(NCH0):
        lo, rows = _chunk0(cix)
        r = 0
        while r < rows:
            w = (lo + r) // S
            seg = min(rows - r, (w + 1) * S - (lo + r))
            # mid-partition writes need DMA (engines can't start at p>0 here)
            nc.gpsimd.dma_start(
                out=mask_all[r:r + seg, cix * WAY + w:cix * WAY + w + 1],
                in_=ones_col_h[0:seg])
            r += seg

    # ---- input loads (params first: they gate the W-fold; spt next;
    # qry per-image so consumers start early) ----
    w_sb = consts.tile([C, C], F32)
    nc.sync.dma_start(out=w_sb[:], in_=conv_w)
    gam_sb = consts.tile([C, 1], F32)
    nc.sync.dma_start(out=gam_sb[:], in_=gam.unsqueeze(1))
    bet_sb = consts.tile([C, 1], F32)
    nc.sync.dma_start(out=bet_sb[:], in_=bet.unsqueeze(1))
    mu_sb = consts.tile([C, 1], F32)
    nc.sync.dma_start(out=mu_sb[:], in_=mu.unsqueeze(1))
    var_sb = consts.tile([C, 1], F32)
    nc.sync.dma_start(out=var_sb[:], in_=var.unsqueeze(1))
    spt_raw = consts.tile([C, SP], F32)
    for w in range(WAY):
        nc.sync.dma_start(out=spt_raw[:, w * S:(w + 1) * S],
                          in_=spt[w])
    qry_raw = consts.tile([C, QPC * S], F32)
    for q in range(QPC):
        nc.sync.dma_start(out=qry_raw[:, q * S:(q + 1) * S],
                          in_=qry[q])
    scale_b = consts.tile([25, 1], F32)
    nc.gpsimd.dma_start(
        out=scale_b[:],
        in_=bass.AP(tensor=scl.tensor, offset=scl.offset, ap=[[0, 25], [1, 1]]))

    # conv weight transposed: lhsT layout [c_in, c_out]
    wT_ps = ps_misc.tile([C, C], F32, tag="m")
    nc.tensor.transpose(wT_ps[:], w_sb[:], ident[0:C, 0:C])
    wT_sb = consts.tile([C, C], F32)
    nc.vector.tensor_copy(wT_sb[:], wT_ps[:])

    # BN fold: bns = gamma / sqrt(var + 1e-5);  bnb = beta - mean * bns
    sd = pre.tile([C, 1], F32, tag="bn")
    nc.scalar.activation(sd[:], var_sb[:], AF.Ln, bias=c1e5[0:C], scale=1.0)
    rsd = pre.tile([C, 1], F32, tag="bn")
    nc.scalar.activation(rsd[:], sd[:], AF.Exp, bias=zeros[0:C], scale=-0.5)
    bns = consts.tile([C, 1], F32)
    nc.vector.tensor_mul(bns[:], gam_sb[:], rsd[:])
    mb = pre.tile([C, 1], F32, tag="bn")
    nc.vector.tensor_mul(mb[:], mu_sb[:], bns[:])
    bnb = consts.tile([C, 1], F32)
    nc.vector.scalar_tensor_tensor(out=bnb[:], in0=mb[:], scalar=-1.0,
                                   in1=bet_sb[:], op0=OP.mult, op1=OP.add)

    # fold the channel-mean centering into the conv weight (rank-1 update):
    # y = W(x - xbar) = W'x with W' = W - wbar 1^T, wbar = W @ ones/64.
    # Keeps the hot loop free of fp32 matmuls (fp32 PE ops hold the clock
    # gate at 1.2 GHz).
    wbar_ps = ps_misc.tile([1, C], F32, tag="m")
    nc.tensor.matmul(out=wbar_ps[:], lhsT=oinv_rep[:, 0:1], rhs=wT_sb[:],
                     start=True, stop=True)
    wbarT = consts.tile([1, C], BF16)
    nc.vector.tensor_copy(wbarT[:], wbar_ps[:])
    wrep_ps = ps_misc.tile([C, C], F32, tag="m")
    nc.tensor.matmul(out=wrep_ps[:], lhsT=ones_h[0:1, 0:C], rhs=wbarT[:],
                     start=True, stop=True)
    wc_h = consts.tile([C, C], BF16)
    nc.vector.tensor_sub(wc_h[:], wT_sb[:], wrep_ps[:])

    # ---- channel-mean centering of the support (for cs-branch pooling) ----
    spt_n = consts.tile([C, SP], F32)
    for w in range(WAY):
        mean_ps = ps_misc.tile([C, S], F32, tag="m")
        nc.tensor.matmul(out=mean_ps[:], lhsT=oinv_rep[:],
                         rhs=spt_raw[:, w * S:(w + 1) * S], start=True, stop=True)
        nc.vector.tensor_sub(spt_n[:, w * S:(w + 1) * S],
                             spt_raw[:, w * S:(w + 1) * S], mean_ps[:])
    # raw support in bf16 (feature-transform input; centering is in W')
    sptb = consts.tile([C, SP], BF16)
    nc.vector.tensor_copy(sptb[:], spt_raw[:])

    # ---- feature transform: conv+bn+relu, L2-normalize over channels, Gram ----
    def feat_transform(x_slice, h_out, gc_out, hT_out):
        """x_slice: [C, S] bf16 RAW features (centering folded into wc_h);
        h_out: [C, S] bf16 AP for the normalized features (channel-major);
        gc_out: [C, C] bf16 AP for the centered Gram; hT_out: [CH1, NCH1*C]
        bf16 tile (position-major, 100-chunks) or None."""
        y_ps = ps_misc.tile([C, S], F32, tag="m")
        nc.tensor.matmul(out=y_ps[:], lhsT=wc_h[:], rhs=x_slice,
                         start=True, stop=True)
        bnr = pre.tile([C, S], BF16, tag="bnr")
        nc.scalar.activation(bnr[:], y_ps[:], AF.Relu, bias=bnb[:], scale=bns[:])
        hT_raw = pre.tile([CH1, NCH1 * C], BF16, tag="hTraw")
        nsq = pre.tile([CH1, NCH1], F32, tag="nsq")
        for j in range(NCH1):
            tp_ps = ps_misc.tile([CH1, C], BF16, tag="m")
            nc.tensor.transpose(tp_ps[:], bnr[:, j * CH1:(j + 1) * CH1],
                                ident_h[0:C, 0:C])
            nc.vector.tensor_copy(hT_raw[:, j * C:(j + 1) * C], tp_ps[:])
            sqscr = pre.tile([CH1, C], F32, tag="sqscr")
            nc.vector.scalar_tensor_tensor(
                out=sqscr[:], in0=hT_raw[:, j * C:(j + 1) * C], scalar=1.0,
                in1=hT_raw[:, j * C:(j + 1) * C],
                op0=OP.mult, op1=OP.mult, accum_out=nsq[:, j:j + 1])
        nc.vector.tensor_scalar_max(nsq[:], nsq[:], 1.0e-16)
        nrm = pre.tile([CH1, NCH1], F32, tag="nrm")
        nc.scalar.activation(nrm[:], nsq[:], AF.Ln, bias=zeros[0:CH1], scale=1.0)
        rinv = pre.tile([CH1, NCH1], F32, tag="rinv")
        nc.scalar.activation(rinv[:], nrm[:], AF.Exp, bias=zeros[0:CH1], scale=-0.5)
        hT_sc = hT_out if hT_out is not None else pre.tile(
            [CH1, NCH1 * C], BF16, tag="hTsc")
        for j in range(NCH1):
            nc.vector.tensor_scalar_mul(hT_sc[:, j * C:(j + 1) * C],
                                        hT_raw[:, j * C:(j + 1) * C], rinv[:, j:j + 1])
        # normalized features back to channel-major + spatial row-sum accum
        rowp = pre.tile([C, NCH1], F32, tag="rowp")
        for j in range(NCH1):
            bk_ps = ps_misc.tile([C, CH1], BF16, tag="m")
            nc.tensor.transpose(bk_ps[:], hT_sc[:, j * C:(j + 1) * C],
                                ident_h[0:CH1, 0:CH1])
            nc.vector.tensor_scalar(
                out=h_out[:, j * CH1:(j + 1) * CH1], in0=bk_ps[:],
                scalar1=1.0, scalar2=None, op0=OP.mult, op1=OP.add,
                accum_out=rowp[:, j:j + 1])
        # centered Gram: Gc = sum_p (h_p - hbar)(h_p - hbar)^T
        hbar = pre.tile([C, 1], F32, tag="hbar")
        nc.vector.tensor_reduce(out=hbar[:], in_=rowp[:], axis=AX.X, op=OP.add)
        hbar_b = pre.tile([C, 1], BF16, tag="hbarb")
        nc.vector.tensor_copy(hbar_b[:], hbar[:])
        hbT_ps = ps_misc.tile([1, C], BF16, tag="m")
        nc.tensor.transpose(hbT_ps[:], hbar_b[:], ident_h[0:C, 0:C])
        hbar_h = pre.tile([1, C], BF16, tag="hbarh")
        nc.vector.tensor_scalar_mul(hbar_h[:], hbT_ps[:], 1.0 / S)
        hb_ps = ps_misc.tile([CH1, C], F32, tag="m")
        nc.tensor.matmul(out=hb_ps[:], lhsT=ones_h[0:1, 0:CH1], rhs=hbar_h[:],
                         start=True, stop=True)
        hTc = pre.tile([CH1, NCH1 * C], BF16, tag="hTc")
        for j in range(NCH1):
            nc.vector.tensor_sub(hTc[:, j * C:(j + 1) * C],
                                 hT_sc[:, j * C:(j + 1) * C], hb_ps[:])
        Gc_ps = ps_misc.tile([C, C], F32, tag="m")
        for j in range(NCH1):
            nc.tensor.matmul(out=Gc_ps[:], lhsT=hTc[:, j * C:(j + 1) * C],
                             rhs=hTc[:, j * C:(j + 1) * C],
                             start=(j == 0), stop=(j == NCH1 - 1))
        nc.vector.tensor_copy(gc_out, Gc_ps[:])

    # ---- support features (once, stage-batched across all 5 ways so the
    # engines overlap instead of walking one way's serial chain at a time) ----
    s_all = consts.tile([C, SP], BF16)            # normalized feats, all ways
    gc_s5 = consts.tile([C, WAY * C], BF16)       # 5 centered Grams
    NB = SP // 500                                # 4 psum-bank batches
    NCK = SP // CH1                               # 20 position chunks
    bnr_all = consts.tile([C, SP], BF16)
    shT_raw = consts.tile([CH1, NCK * C], BF16)
    nsq_all = consts.tile([CH1, NCK], F32)
    shT_sc = consts.tile([CH1, NCK * C], BF16)
    srowp = consts.tile([C, NCK // 2], F32)
    shTc = consts.tile([CH1, NCK * C], BF16)
    hT_all = consts.tile([128, NCH0 * C], BF16)

    def support_stage1():
        for b in range(NB):
            y_ps = ps_misc.tile([C, 500], F32, tag="m", name="y_ps")
            nc.tensor.matmul(out=y_ps[:], lhsT=wc_h[:],
                             rhs=sptb[:, b * 500:(b + 1) * 500],
                             start=True, stop=True)
            nc.scalar.activation(bnr_all[:, b * 500:(b + 1) * 500], y_ps[:],
                                 AF.Relu, bias=bnb[:], scale=bns[:])

    def support_stage2():
        for jp in range(NCK // 2):
            tp_ps = ps_misc.tile([CH1, 2 * C], BF16, tag="m", name="tp_ps")
            for h in range(2):
                j = 2 * jp + h
                nc.tensor.transpose(tp_ps[:, h * C:(h + 1) * C],
                                    bnr_all[:, j * CH1:(j + 1) * CH1],
                                    ident_h[0:C, 0:C])
            nc.vector.tensor_copy(shT_raw[:, 2 * jp * C:(2 * jp + 2) * C],
                                  tp_ps[:])
            for h in range(2):
                j = 2 * jp + h
                sqscr = pre.tile([CH1, C], F32, tag="sqscr", name="sqscr")
                nc.vector.scalar_tensor_tensor(
                    out=sqscr[:], in0=shT_raw[:, j * C:(j + 1) * C], scalar=1.0,
                    in1=shT_raw[:, j * C:(j + 1) * C],
                    op0=OP.mult, op1=OP.mult, accum_out=nsq_all[:, j:j + 1])
        nc.vector.tensor_scalar_max(nsq_all[:], nsq_all[:], 1.0e-16)
        snrm = pre.tile([CH1, NCK], F32, tag="snrm", name="snrm")
        nc.scalar.activation(snrm[:], nsq_all[:], AF.Ln, bias=zeros[0:CH1],
                             scale=1.0)
        srinv = pre.tile([CH1, NCK], F32, tag="srinv", name="srinv")
        nc.scalar.activation(srinv[:], snrm[:], AF.Exp, bias=zeros[0:CH1],
                             scale=-0.5)
        for j in range(NCK):
            nc.vector.tensor_scalar_mul(shT_sc[:, j * C:(j + 1) * C],
                                        shT_raw[:, j * C:(j + 1) * C],
                                        srinv[:, j:j + 1])

    def support_stage3():
        for jp in range(NCK // 2):
            bk_ps = ps_misc.tile([C, 2 * CH1], BF16, tag="m", name="bk_ps")
            for h in range(2):
                j = 2 * jp + h
                nc.tensor.transpose(bk_ps[:, h * CH1:(h + 1) * CH1],
                                    shT_sc[:, j * C:(j + 1) * C],
                                    ident_h[0:CH1, 0:CH1])
            # pairs are way-pure (4 chunks per way): accumulate pair rowsum
            nc.vector.tensor_scalar(
                out=s_all[:, 2 * jp * CH1:(2 * jp + 2) * CH1], in0=bk_ps[:],
                scalar1=1.0, scalar2=None, op0=OP.mult, op1=OP.add,
                accum_out=srowp[:, jp:jp + 1])
        # per-way spatial means -> centered chunks -> Grams
        hbar5 = pre.tile([C, WAY], F32, tag="hbar5", name="hbar5")
        nc.vector.tensor_reduce(out=hbar5[:],
                                in_=_ap3(srowp[:], [[2, WAY], [1, 2]]),
                                axis=AX.X, op=OP.add)
        hbar5b = pre.tile([C, WAY], BF16, tag="hbar5b", name="hbar5b")
        nc.vector.tensor_copy(hbar5b[:], hbar5[:])
        hbT5 = pre.tile([1, WAY * C], BF16, tag="hbT5", name="hbT5")
        for w in range(WAY):
            hbT_ps = ps_misc.tile([1, C], BF16, tag="m", name="hbT_ps")
            nc.tensor.transpose(hbT_ps[:], hbar5b[:, w:w + 1], ident_h[0:C, 0:C])
            nc.vector.tensor_scalar_mul(hbT5[:, w * C:(w + 1) * C], hbT_ps[:],
                                        1.0 / S)
        for w in range(WAY):
            hb_ps = ps_misc.tile([CH1, C], F32, tag="m", name="hb_ps")
            nc.tensor.matmul(out=hb_ps[:], lhsT=ones_h[0:1, 0:CH1],
                             rhs=hbT5[:, w * C:(w + 1) * C], start=True, stop=True)
            for k in range(NCH1):
                j = w * NCH1 + k
                nc.vector.tensor_sub(shTc[:, j * C:(j + 1) * C],
                                     shT_sc[:, j * C:(j + 1) * C], hb_ps[:])
        for w in range(WAY):
            Gc_ps = ps_misc.tile([C, C], F32, tag="m", name="Gc_ps")
            for k in range(NCH1):
                j = w * NCH1 + k
                nc.tensor.matmul(out=Gc_ps[:], lhsT=shTc[:, j * C:(j + 1) * C],
                                 rhs=shTc[:, j * C:(j + 1) * C],
                                 start=(k == 0), stop=(k == NCH1 - 1))
            nc.vector.tensor_copy(gc_s5[:, w * C:(w + 1) * C], Gc_ps[:])
        # position-major support features, 128-chunks across ways (cq stt)
        for cp in range(NCH0 // 2):
            tp = ps_misc.tile([128, 2 * C], BF16, tag="m", name="tp")
            rows2 = []
            for h in range(2):
                cix = 2 * cp + h
                lo, rows = _chunk0(cix)
                rows2.append(rows)
                nc.tensor.transpose(tp[0:rows, h * C:(h + 1) * C],
                                    s_all[:, lo:lo + rows], ident_h[0:C, 0:C])
            if rows2[0] == rows2[1]:
                nc.vector.tensor_copy(
                    hT_all[0:rows2[0], 2 * cp * C:(2 * cp + 2) * C], tp[0:rows2[0]])
            else:
                for h in range(2):
                    cix = 2 * cp + h
                    nc.vector.tensor_copy(
                        hT_all[0:rows2[h], cix * C:(cix + 1) * C],
                        tp[0:rows2[h], h * C:(h + 1) * C])

    # attention/pooling accumulators over all pairs
    sall = consts.tile([C, PAIRS], F32)   # spt_att columns (pair-major q*5+w)
    qall = consts.tile([C, PAIRS], F32)   # qry_att columns
    sims = consts.tile([25, 2], F32)   # col h = pairs h*25..h*25+24
    attqT = consts.tile([128, 4 * WAY], BF16)     # attn_q^T chunks (per query)
    nc.gpsimd.memset(attqT[:], 0.0)

    # per-query persistent feature arrays (all 10 queries computed upfront,
    # interleaved with the 5 support transforms so every engine stays busy
    # during the startup phase and the hot loop stays pure corr->exp->attn)
    qh_all = consts.tile([C, QPC * S], BF16)
    gcq_all = consts.tile([C, QPC * C], BF16)
    qhT_all = consts.tile([CH1, QPC * NCH1 * C], BF16)
    qnT_all = consts.tile([128, QPC * 4 * C], BF16)

    # ---------------- per-query pipeline ----------------
    qctx = {}

    def emit_feat(q):
        d = {}
        # raw bf16 features, 512-padded for the DMA transpose
        qrb = pre.tile([C, 512], BF16, tag="qrb", name="qrb")
        nc.gpsimd.memset(qrb[:, S:512], 0.0)
        nc.vector.tensor_copy(qrb[:, 0:S], qry_raw[:, q * S:(q + 1) * S])
        d["qh"] = qh_all[:, q * S:(q + 1) * S]
        d["gcq"] = gcq_all[:, q * C:(q + 1) * C]
        d["qhT"] = qhT_all[:, q * NCH1 * C:(q + 1) * NCH1 * C]
        feat_transform(qrb[:, 0:S], d["qh"], d["gcq"], d["qhT"])
        # position-major raw features via DMA transpose, then channel-mean
        # centering per position (row) on DVE: qnT = qnT_raw - rowmean.
        qnTr = pre.tile([128, 4 * C], BF16, tag="qnTr", name="qnTr")
        for j in range(4):
            nc.sync.dma_start_transpose(qnTr[:, j * C:(j + 1) * C],
                                        qrb[:, j * 128:(j + 1) * 128])
        d["qnT"] = qnT_all[:, q * 4 * C:(q + 1) * 4 * C]
        qmean = pre.tile([128, 4], F32, tag="qmean", name="qmean")
        for j in range(4):
            nc.vector.tensor_reduce(out=qmean[:, j:j + 1],
                                    in_=qnTr[:, j * C:(j + 1) * C],
                                    axis=AX.X, op=OP.add)
        nc.vector.tensor_scalar_mul(qmean[:], qmean[:], 1.0 / C)
        for j in range(4):
            nc.vector.tensor_scalar(
                out=d["qnT"][:, j * C:(j + 1) * C],
                in0=qnTr[:, j * C:(j + 1) * C],
                scalar1=qmean[:, j:j + 1], scalar2=None,
                op0=OP.subtract, op1=OP.bypass)
        qctx[q] = d

    def emit_uvar(q):
        """All 36 variance quadratic forms + batched rr for query q.
        stt layout: [128, NCH0 + WAY*NCH1]; cols 0:16 = cq chunks (rows =
        chunk rows), cols 16:36 = cs (way,chunk) (rows 0:100)."""
        d = qctx[q]
        stt = perq.tile([128, NUNITS], F32, tag="stt", name="stt")
        nc.gpsimd.memset(stt[:], 0.0)
        for cix in range(NCH0):
            lo, rows = _chunk0(cix)
            u_ps = ps_misc.tile([128, C], F32, tag="m")
            nc.tensor.matmul(out=u_ps[0:rows], lhsT=s_all[:, lo:lo + rows],
                             rhs=d["gcq"], start=True, stop=True)
            scr = pre.tile([128, C], F32, tag="uscr")
            nc.vector.scalar_tensor_tensor(
                out=scr[0:rows], in0=u_ps[0:rows], scalar=1.0,
                in1=hT_all[0:rows, cix * C:(cix + 1) * C],
                op0=OP.mult, op1=OP.mult, accum_out=stt[0:rows, cix:cix + 1])
        qhTq = d["qhT"]
        for j in range(NCH1):
            u5_ps = ps_misc.tile([CH1, WAY * C], F32, tag="m")
            nc.tensor.matmul(out=u5_ps[:],
                             lhsT=d["qh"][:, j * CH1:(j + 1) * CH1],
                             rhs=gc_s5[:], start=True, stop=True)
            for w in range(WAY):
                scr = pre.tile([CH1, C], F32, tag="uscr")
                col = NCH0 + j * WAY + w
                nc.vector.scalar_tensor_tensor(
                    out=scr[:], in0=u5_ps[:, w * C:(w + 1) * C], scalar=1.0,
                    in1=qhTq[:, j * C:(j + 1) * C],
                    op0=OP.mult, op1=OP.mult, accum_out=stt[0:CH1, col:col + 1])
        # rr = 1/(TEMP*sqrt(var+1e-5)), var = stt/399; junk rows stay finite.
        sq = perq.tile([128, NUNITS], F32, tag="sq", name="sq")
        nc.scalar.activation(sq[:], stt[:], AF.Ln, bias=c25e5[:],
                             scale=(TEMP * TEMP) / (S - 1.0))
        rr = perq.tile([128, NUNITS], F32, tag="rr", name="rr")
        nc.scalar.activation(rr[:], sq[:], AF.Exp, bias=zeros[:], scale=-0.5)
        d["rr"] = rr
        d["z"] = perq.tile([128, NUNITS], F32, tag="z", name="z")
        d["E0"] = [None] * NCH0
        d["E1"] = [None] * (WAY * NCH1)

    # unit list: interleave cq chunks and cs (way, chunk) units
    units = []
    u0 = [("cq", cix) for cix in range(NCH0)]
    u1 = [("cs", w * NCH1 + k) for w in range(WAY) for k in range(NCH1)]
    i0 = i1 = 0
    for i in range(NUNITS):
        # ratio 16:20 -> alternate with slight cs surplus
        if (i * NCH0) // NUNITS >= i0 + (1 if i1 > i0 else 0) and i0 < NCH0:
            units.append(u0[i0]); i0 += 1
        elif i1 < len(u1):
            units.append(u1[i1]); i1 += 1
        else:
            units.append(u0[i0]); i0 += 1

    def emit_unit(q, ui):
        """One corr matmul + exp (+ denominator) unit."""
        d = qctx[q]
        kind, ix = units[ui]
        on_dve = ui < N_DENOM_DVE  # spread: first units' denoms on DVE
        if kind == "cq":
            cix = ix
            lo, rows = _chunk0(cix)
            cp = ps_corr.tile([128, S], F32, tag="corr")
            nc.tensor.matmul(out=cp[0:rows], lhsT=s_all[:, lo:lo + rows],
                             rhs=d["qh"][:], start=True, stop=True)
            e = e0pool.tile([128, S], BF16, tag="E0", name="E0")
            col = cix
            zcol = d["z"][0:rows, col:col + 1]
            if on_dve:
                nc.scalar.activation(e[0:rows], cp[0:rows], AF.Exp,
                                     bias=zeros[0:rows],
                                     scale=d["rr"][0:rows, col:col + 1])
                nc.vector.tensor_reduce(out=zcol, in_=e[0:rows], axis=AX.X,
                                        op=OP.add)
            else:
                nc.scalar.activation(e[0:rows], cp[0:rows], AF.Exp,
                                     bias=zeros[0:rows],
                                     scale=d["rr"][0:rows, col:col + 1],
                                     accum_out=zcol)
            d["E0"][cix] = e
        else:
            w, k = ix // NCH1, ix % NCH1
            cp = ps_corr.tile([128, S], F32, tag="corr")
            nc.tensor.matmul(out=cp[0:CH1],
                             lhsT=d["qh"][:, k * CH1:(k + 1) * CH1],
                             rhs=s_all[:, w * S:(w + 1) * S], start=True, stop=True)
            e = e1pool.tile([CH1, S], BF16, tag="E1", name="E1")
            col = NCH0 + k * WAY + w
            zcol = d["z"][0:CH1, col:col + 1]
            if on_dve:
                nc.scalar.activation(e[:], cp[0:CH1], AF.Exp, bias=zeros[0:CH1],
                                     scale=d["rr"][0:CH1, col:col + 1])
                nc.vector.tensor_reduce(out=zcol, in_=e[:], axis=AX.X, op=OP.add)
            else:
                nc.scalar.activation(e[:], cp[0:CH1], AF.Exp, bias=zeros[0:CH1],
                                     scale=d["rr"][0:CH1, col:col + 1],
                                     accum_out=zcol)
            d["E1"][ix] = e

    def tail_ops(q):
        """Attention + pooling for query q as a list of thunks (emitted
        interleaved into the next query's exp stream)."""
        d = qctx[q]
        ops = []

        def recips():
            wrec = perq.tile([128, NUNITS], BF16, tag="wrec", name="wrec")
            with nc.allow_low_precision(reason="bf16 softmax weights"):
                nc.vector.reciprocal(wrec[:], d["z"][:])
            d["wrec"] = wrec
            # masked cq attention weights in ONE stride-0 bcast op:
            # mw[p, c, w] = mask_all[p, c, w] * wrec[p, c]
            mw = perq.tile([128, NCH0 * WAY], BF16, tag="mw", name="mw")
            nc.vector.tensor_mul(
                _ap3(mw[:], [[WAY, NCH0], [1, WAY]]),
                _ap3(mask_all[:], [[WAY, NCH0], [1, WAY]]),
                _ap3(wrec[:], [[1, NCH0], [0, WAY]]))
            d["mw"] = mw
        ops.append(recips)

        # cq attention: 16 masked matmuls accumulate attn_q [5, 400]
        attq = {}

        def mk_cq_attn(cix, first, last):
            def f():
                if first:
                    attq["t"] = ps_attq.tile([WAY, S], F32, tag="attq", name="attq")
                lo, rows = _chunk0(cix)
                nc.tensor.matmul(
                    out=attq["t"][:],
                    lhsT=d["mw"][0:rows, cix * WAY:(cix + 1) * WAY],
                    rhs=d["E0"][cix][0:rows], start=first, stop=last)
            return f
        for cix in range(NCH0):
            ops.append(mk_cq_attn(cix, cix == 0, cix == NCH0 - 1))

        def attq_tail():
            # attn_q -> SBUF bf16, transpose 128-pieces, pool via PE
            aq = perq.tile([WAY, S], BF16, tag="aq", name="aq")
            nc.vector.tensor_copy(aq[:], attq["t"][:])
            d["aq"] = aq
        ops.append(attq_tail)

        def mk_attq_tp(j):
            def f():
                npc = min(128, S - j * 128)
                tp = ps_misc.tile([128, WAY], BF16, tag="m")
                nc.tensor.transpose(tp[0:npc], d["aq"][:, j * 128:j * 128 + npc],
                                    ident_h[0:WAY, 0:WAY])
                nc.vector.tensor_copy(attqT[0:npc, j * WAY:(j + 1) * WAY],
                                      tp[0:npc])
            return f
        for j in range(4):
            ops.append(mk_attq_tp(j))

        qatt = {}

        def mk_qpool(j, first, last):
            def f():
                if first:
                    qatt["t"] = ps_misc.tile([C, WAY], F32, tag="m", name="qatt")
                npc = min(128, S - j * 128)
                nc.tensor.matmul(out=qatt["t"][:],
                                 lhsT=qctx[q]["qnT"][0:npc, j * C:(j + 1) * C],
                                 rhs=attqT[0:npc, j * WAY:(j + 1) * WAY],
                                 start=first, stop=last)
                if last:
                    nc.vector.tensor_copy(qall[:, q * WAY:(q + 1) * WAY],
                                          qatt["t"][:])
            return f
        for j in range(4):
            ops.append(mk_qpool(j, j == 0, j == 3))

        # cs attention per way: bcast-column 1/Z lhsT, pool on DVE
        atts = {}

        def mk_cs_attn(w, k, first, last):
            def f():
                if first:
                    atts["t"] = ps_atts.tile([C, S], F32, tag="atts", name="atts")
                col = NCH0 + k * WAY + w
                nc.tensor.matmul(
                    out=atts["t"][:],
                    lhsT=_bcast_col(d["wrec"][0:CH1, col:col + 1], C),
                    rhs=d["E1"][w * NCH1 + k][:], start=first, stop=last)
                if last:
                    pj = pre.tile([C, S], F32, tag="pj")
                    nc.vector.scalar_tensor_tensor(
                        out=pj[:], in0=spt_n[:, w * S:(w + 1) * S],
                        scalar=1.0, in1=atts["t"][:], op0=OP.mult, op1=OP.mult,
                        accum_out=sall[:, q * WAY + w:q * WAY + w + 1])
            return f
        for w in range(WAY):
            for k in range(NCH1):
                ops.append(mk_cs_attn(w, k, k == 0, k == NCH1 - 1))

        def cleanup():
            qctx.pop(q, None)
        ops.append(cleanup)
        return ops

    def emit_cosine(plo, phi):
        n = phi - plo
        h = plo // 25
        p3 = pre.tile([C, 3 * PAIRS], F32, tag="p3", name="p3")
        nc.vector.tensor_mul(p3[:, 0:n], sall[:, plo:phi], qall[:, plo:phi])
        nc.vector.tensor_mul(p3[:, n:2 * n], sall[:, plo:phi], sall[:, plo:phi])
        nc.vector.tensor_mul(p3[:, 2 * n:3 * n], qall[:, plo:phi], qall[:, plo:phi])
        dots = ps_misc.tile([PAIRS, 3], F32, tag="m", name="dots")
        for i in range(3):
            nc.tensor.matmul(out=dots[0:n, i:i + 1],
                             lhsT=p3[:, i * n:(i + 1) * n],
                             rhs=ones128[0:C], start=True, stop=True)
        nrm2 = pre.tile([PAIRS, 2], F32, tag="nrm2", name="nrm2")
        nc.vector.tensor_scalar_max(nrm2[0:n], dots[0:n, 1:3], 1.6e-7)
        lnn = pre.tile([PAIRS, 2], F32, tag="lnn", name="lnn")
        nc.scalar.activation(lnn[0:n], nrm2[0:n], AF.Ln, bias=zeros[0:n], scale=1.0)
        lsum = pre.tile([PAIRS, 1], F32, tag="lsum", name="lsum")
        nc.vector.tensor_add(lsum[0:n], lnn[0:n, 0:1], lnn[0:n, 1:2])
        rden = pre.tile([PAIRS, 1], F32, tag="rden", name="rden")
        nc.scalar.activation(rden[0:n], lsum[0:n], AF.Exp, bias=zeros[0:n], scale=-0.5)
        s0 = pre.tile([PAIRS, 1], F32, tag="s0", name="s0")
        nc.vector.tensor_mul(s0[0:n], dots[0:n, 0:1], rden[0:n])
        nc.vector.tensor_mul(sims[0:n, h:h + 1], s0[0:n], scale_b[0:n])

    # ---------------- top-level schedule ----------------
    support_stage1()
    emit_feat(0)
    support_stage2()
    emit_feat(1)
    support_stage3()
    emit_uvar(0)
    pending = []           # tail thunks of the previous query
    for q in range(QPC):
        for ui in range(NUNITS):
            emit_unit(q, ui)
            # spread previous query's attention/pool tail into this stream
            take = (len(pending) + NUNITS - 1 - ui) // (NUNITS - ui)
            for _ in range(take):
                pending.pop(0)()
            # next-query prep at fixed points
            if q + 2 < QPC and ui == 8:
                emit_feat(q + 2)
            if q + 1 < QPC and ui == 22:
                emit_uvar(q + 1)
        pending = tail_ops(q)
        if q == 5:
            pending.append(lambda: emit_cosine(0, 25))
    for f in pending:
        f()
    emit_cosine(25, PAIRS)
    nc.sync.dma_start(
        out=bass.AP(tensor=out_sim.tensor, offset=out_sim.offset,
                    ap=[[1, 25], [25, 2]]),
        in_=sims[:])


_PROGRAM = None


def _get_program():
    global _PROGRAM
    if _PROGRAM is None:
        _PROGRAM = _build_program()
    return _PROGRAM


def kernel(spt, qry, conv_w, bn_gamma, bn_beta, bn_mean, bn_var, scale):
    spt = np.ascontiguousarray(np.asarray(spt, dtype=np.float32).reshape(WAY, C, S))
    qry = np.asarray(qry, dtype=np.float32).reshape(-1, C, S)
    nq = qry.shape[0]
    npad = NCORES * QPC
    qpad = np.zeros((npad, C, S), dtype=np.float32)
    qpad[:nq] = qry
    in_maps = []
    for core in range(NCORES):
        in_maps.append({
            "spt": spt,
            "qry": np.ascontiguousarray(qpad[core * QPC:(core + 1) * QPC]),
            "conv_w": np.asarray(conv_w, dtype=np.float32),
            "bn_gamma": np.asarray(bn_gamma, dtype=np.float32),
            "bn_beta": np.asarray(bn_beta, dtype=np.float32),
            "bn_mean": np.asarray(bn_mean, dtype=np.float32),
            "bn_var": np.asarray(bn_var, dtype=np.float32),
            "scale": np.asarray(scale, dtype=np.float32),
        })
    nc = _get_program()
    trace = bool(os.environ.get("KBENCH_TRACE"))
    kw = {}
    if trace:
        import tempfile
        kw = dict(trace=True, tmpdir=tempfile.mkdtemp(prefix="ktrace_"))
    res = run_bass_kernel_spmd(nc, in_maps, list(range(NCORES)), **kw)
    if trace:
        global LAST_RESULTS
        LAST_RESULTS = res
        print("exec_time_ns:", res.exec_time_ns,
              "mean:", res.mean_exec_time_ns,
              "worst core:", res.max_exec_time_core_id)
        if res.instructions_and_trace:
            print("trace path:", res.instructions_and_trace[1])
    outs = [np.asarray(res.results[i]["out_sim"]) for i in range(NCORES)]
    full = np.concatenate([o.reshape(QPC, WAY) for o in outs], axis=0)  # [80, 5]
    return np.ascontiguousarray(full[:nq]).astype(np.float32)


if __name__ == "__main__":
    rng = np.random.default_rng(0)
    ins = {
        "spt": rng.standard_normal((WAY, C, 20, 20), dtype=np.float32),
        "qry": rng.standard_normal((75, C, 20, 20), dtype=np.float32),
        "conv_w": (rng.standard_normal((C, C)) * 0.1).astype(np.float32),
        "bn_gamma": np.ones(C, np.float32),
        "bn_beta": np.zeros(C, np.float32),
        "bn_mean": np.zeros(C, np.float32),
        "bn_var": np.ones(C, np.float32),
        "scale": np.ones(1, np.float32),
    }
    out = kernel(**ins)
    print(out.shape, out.dtype, out[:2])
